# revision 1
# baseline (speedup 1.0000x reference)
"""Trainium2 Bass kernel for a 3-layer edge-featured GAT over 256 dense 84-node graphs.

Contract: kernel(**inputs) takes the FULL unsharded inputs (as produced by the
problem's setup_inputs) and returns the FULL [256, 1] float32 output.

Strategy (data parallel over graphs, 32 graphs/core on 8 cores):
  Each graph is dense (all ordered pairs + self loops), so message passing
  collapses to dense per-graph [84, 84] attention matrices. Host-side we
  scatter edge_attr into dense per-graph planes (folding the per-layer edge
  MLP down to a scalar per edge, and the PyG mean self-loop attr onto the
  diagonal), fold a_src/a_dst/readout into augmented layer weights, and keep
  a constant-one input feature so every projection carries a ones column
  (which turns the softmax denominator into one extra matmul column).

  Per layer on device: one combined projection produces, per node, the
  projected features h~, a_src/a_dst attention scalars and a constant 1;
  the [src, dst] logit plane is accumulated in PSUM from the host E plane
  (identity matmul), a rank-1 broadcast of a_dst, and small per-chunk
  block-diagonal mask matmuls broadcasting a_src; exp(lrelu(x)) is computed
  as max(exp(x), exp(0.2 x)) (two ScalarE exps off PSUM + one cheap fp16 DVE
  max); per-graph matmuls of ex_g against node-major [h~ | 1] give aggregate
  + denominator in one pass; relu and the 1/den normalization fuse into one
  strided scalar_tensor_tensor per PSUM bank with a stride-0 broadcast AP.

  All matmul operands are fp16 (PSUM accumulation stays fp32): fp32 matmuls
  on TRN2 run 2-pass LOW_HIGH at 4 cycles/row, fp16 runs 1 cycle/row.
  Measured end-to-end error of the fp16 config vs the fp32 reference:
  ~3.5e-4 scale-relative.
"""

import sys

for _p in ("/opt/trn_rl_repo",):
    if _p not in sys.path:
        sys.path.append(_p)

import numpy as np

from contextlib import ExitStack

from concourse import bacc, bass, mybir, tile
from concourse.bass_types import AP
from concourse.bass_utils import run_bass_kernel_spmd

F32 = mybir.dt.float32
F16 = mybir.dt.float16
AF = mybir.ActivationFunctionType
ALU = mybir.AluOpType

NPG = 84            # nodes per graph
B = 256             # graphs
HID = 64
DEPTH = 3
NEG_SLOPE = 0.2
NC_CORES = 8
GPC = B // NC_CORES     # 32 graphs per core
NB = GPC * NPG          # 2688 nodes per core
CH = 448                # free-dim chunk (one PSUM bank)
NCH = NB // CH          # 6 chunks

# projection column layout: [a_dst | W(64) | ones | a_src | v(layer2)]
C_ADST, C_W0, C_ONE, C_ASRC, C_V = 0, 1, 65, 66, 67


def _chunk_graphs(c):
    """Graphs whose columns intersect chunk c."""
    g_lo = (CH * c) // NPG
    g_hi = (CH * (c + 1) - 1) // NPG
    return g_lo, min(g_hi, GPC - 1)


def _host_preprocess(inputs):
    x = np.ascontiguousarray(np.asarray(inputs['x'], np.float32))
    ei = np.asarray(inputs['edge_index'])
    ea = np.asarray(inputs['edge_attr'], np.float32)
    W0 = np.asarray(inputs['W0'], np.float32)
    Ws = np.asarray(inputs['Ws'], np.float32)
    asl = np.asarray(inputs['att_src_all'], np.float32)
    adl = np.asarray(inputs['att_dst_all'], np.float32)
    Wel = np.asarray(inputs['W_edge_all'], np.float32)
    ael = np.asarray(inputs['att_edge_all'], np.float32)
    bl = np.asarray(inputs['bias_all'], np.float32)
    linW = np.asarray(inputs['lin_W'], np.float32)
    linb = np.asarray(inputs['lin_b'], np.float32)

    src, dst = np.asarray(ei[0]), np.asarray(ei[1])
    g = src // NPG
    assert np.all(dst // NPG == g), "edges cross graph boundaries"
    sl, dl = src % NPG, dst % NPG

    dense = np.zeros((B, NPG, NPG, 2), np.float32)
    dense[g, sl, dl] = ea
    cnt = np.zeros((B, NPG), np.float32)
    np.add.at(cnt, (g, dl), 1.0)
    colsum = dense.sum(axis=1)
    loop_attr = colsum / np.maximum(cnt, 1.0)[..., None]
    di = np.arange(NPG)
    dense[:, di, di, :] = loop_attr

    Es = []
    for l in range(DEPTH):
        w2 = Wel[l] @ ael[l]
        Es.append(np.ascontiguousarray(dense @ w2, dtype=np.float16))

    W_all = [W0, Ws[0], Ws[1]]
    CW = []
    for l in range(DEPTH):
        K = W_all[l].shape[0]
        cols = [(W_all[l] @ adl[l])[:, None], W_all[l], np.zeros((K, 1), np.float32),
                (W_all[l] @ asl[l])[:, None]]
        if l == DEPTH - 1:
            cols.append(W_all[l] @ linW)
        A = np.concatenate(cols, axis=1)
        aug = np.zeros((1, A.shape[1]), np.float32)
        aug[0, C_ONE] = 1.0
        CW.append(np.ascontiguousarray(np.vstack([A, aug]), np.float16))

    tail_bias = float(NPG * float(bl[DEPTH - 1] @ linW[:, 0]) + float(linb[0]))

    # per-chunk block-diagonal masks: row k of chunk c covers graph g_lo(c)+k
    maskc = np.zeros((7, NB), np.float16)
    for c in range(NCH):
        g_lo, _ = _chunk_graphs(c)
        for j in range(CH):
            gg = (CH * c + j) // NPG
            maskc[gg - g_lo, CH * c + j] = 1.0
    # merged-logits stationary: per chunk-column-block, rows 0..83 identity,
    # rows 84..90 runtime a_src rows, row 91 ones (pairs with the a_dst row)
    lhs92 = np.zeros((92, NCH * NPG), np.float16)
    for c in range(NCH):
        lhs92[0:NPG, c * NPG:(c + 1) * NPG] = np.eye(NPG, dtype=np.float16)
    lhs92[91, :] = 1.0
    ident = np.eye(NPG, dtype=np.float16)
    x_aug = np.ones((2, B * NPG), np.float16)
    x_aug[0] = x[:, 0].astype(np.float16)

    return dict(x_aug=x_aug, Es=Es, CW=CW, bl=bl, tail_bias=tail_bias,
                maskc=maskc, lhs92=lhs92, ident=ident)


def _graph_banks(n_graphs, per_bank):
    out = []
    g0 = 0
    while g0 < n_graphs:
        out.append(list(range(g0, min(g0 + per_bank, n_graphs))))
        g0 += per_bank
    return out


def _bcast_inner(ap, n):
    """View `ap` with an extra innermost stride-0 axis of length n."""
    return AP(ap.tensor, ap.offset, list(ap.ap) + [[0, n]])


def _build_program(tail_bias, use_bias):
    """use_bias: (bool, bool) for layers 0 and 1 (per-node bias via ex@bb matmul)."""
    nc = bacc.Bacc("TRN2", target_bir_lowering=False, debug=False)

    xT_d = nc.dram_tensor("xT", [2, NB], F16, kind="ExternalInput").ap()
    E_d = [nc.dram_tensor(f"E{l}", [NPG, NB], F16, kind="ExternalInput").ap()
           for l in range(DEPTH)]
    ncw = [67, 67, 68]
    CW_d = [nc.dram_tensor(f"CW{l}", [(2 if l == 0 else HID + 1), ncw[l]],
                           F16, kind="ExternalInput").ap() for l in range(DEPTH)]
    maskc_d = nc.dram_tensor("maskc", [7, NB], F16, kind="ExternalInput").ap()
    lhs92_d = nc.dram_tensor("lhs92", [92, NCH * NPG], F16, kind="ExternalInput").ap()
    ident_d = nc.dram_tensor("ident", [NPG, NPG], F16, kind="ExternalInput").ap()
    bb_d = [nc.dram_tensor(f"bb{l}", [NPG, HID], F16, kind="ExternalInput").ap()
            if use_bias[l] else None for l in range(2)]
    # row bounce scratch (sbuf row -> dram -> repartitioned sbuf)
    asrc_tmp = [nc.dram_tensor(f"asrc_tmp{l}", [NB], F16).ap() for l in range(DEPTH)]
    v_tmp = nc.dram_tensor("v_tmp", [NB], F16).ap()
    q_tmp = nc.dram_tensor("q_tmp", [NPG * GPC], F32).ap()
    out_d = nc.dram_tensor("out", [GPC], F32, kind="ExternalOutput").ap()

    with tile.TileContext(nc) as tc, ExitStack() as ctx:
        cpool = ctx.enter_context(tc.tile_pool(name="const", bufs=1))
        hpool = ctx.enter_context(tc.tile_pool(name="h", bufs=2))
        ppool = ctx.enter_context(tc.tile_pool(name="proj", bufs=2))
        npool = ctx.enter_context(tc.tile_pool(name="hnode", bufs=2))
        expool = ctx.enter_context(tc.tile_pool(name="ex", bufs=2))
        echpool = ctx.enter_context(tc.tile_pool(name="ech", bufs=4))
        ltpool = ctx.enter_context(tc.tile_pool(name="lt", bufs=3))
        smpool = ctx.enter_context(tc.tile_pool(name="small", bufs=3))
        upool = ctx.enter_context(tc.tile_pool(name="u", bufs=2))

        psb = ctx.enter_context(tc.tile_pool(name="psb", bufs=4, space="PSUM"))
        psa = ctx.enter_context(tc.tile_pool(name="psa", bufs=2, space="PSUM"))
        pst = ctx.enter_context(tc.tile_pool(name="pst", bufs=1, space="PSUM"))
        pst2 = ctx.enter_context(tc.tile_pool(name="pst2", bufs=1, space="PSUM"))

        # constants
        lhs92_sb = cpool.tile([92, NCH * NPG], F16, tag="lhs92")
        nc.sync.dma_start(lhs92_sb[:], lhs92_d[:])
        ident_sb = cpool.tile([NPG, NPG], F16, tag="ident")
        nc.sync.dma_start(ident_sb[:], ident_d[:])
        cw_sb = []
        for l in range(DEPTH):
            t = cpool.tile(list(CW_d[l].shape), F16, tag=f"cw{l}")
            nc.sync.dma_start(t[:], CW_d[l][:])
            cw_sb.append(t)
        bb_sb = []
        for l in range(2):
            if use_bias[l]:
                t = cpool.tile([NPG, HID], F16, tag=f"bb{l}")
                nc.sync.dma_start(t[:], bb_d[l][:])
                bb_sb.append(t)
            else:
                bb_sb.append(None)

        x_sb = hpool.tile([2, NB], F16, tag="x")
        nc.sync.dma_start(x_sb[:], xT_d[:])


        def projection_and_logits(l, hT_in, nrows):
            """Returns (pT, ex): projection tile [nrows, NB] and attention ex."""
            pT = ppool.tile([nrows, NB], F16, tag="pT")
            for c in range(NCH):
                cs = slice(c * CH, (c + 1) * CH)
                pw = psb.tile([nrows, CH], F32, tag="pb")
                nc.tensor.matmul(pw[:], cw_sb[l][:], hT_in[:, cs],
                                 start=True, stop=True)
                if c % 2 == 0:
                    nc.scalar.copy(pT[:, cs], pw[:])
                else:
                    nc.vector.tensor_copy(pT[:, cs], pw[:])
                # bounce this chunk's a_src row immediately
                nc.sync.dma_start(
                    asrc_tmp[l][cs].rearrange("(o n) -> o n", o=1),
                    pT[C_ASRC:C_ASRC + 1, cs])
            for c in range(NCH):
                g_lo, g_hi = _chunk_graphs(c)
                ng = g_hi - g_lo + 1
                nc.sync.dma_start(
                    lhs92_sb[84:84 + ng, c * NPG:(c + 1) * NPG],
                    asrc_tmp[l][g_lo * NPG:(g_hi + 1) * NPG]
                    .rearrange("(g s) -> g s", g=ng))
            ex = expool.tile([NPG, NB], F16, tag="ex")
            for c in range(NCH):
                cs = slice(c * CH, (c + 1) * CH)
                # assemble moving operand: [E(84) | maskc(7) | a_dst(1)]
                ech = echpool.tile([92, CH], F16, tag="ech")
                nc.sync.dma_start(ech[0:NPG, :], E_d[l][:, cs])
                nc.gpsimd.dma_start(ech[NPG:NPG + 7, :], maskc_d[:, cs])
                nc.sync.dma_start(ech[91:92, :], pT[C_ADST:C_ADST + 1, cs])
                pl = psb.tile([NPG, CH], F32, tag="pb")
                nc.tensor.matmul(pl[:], lhs92_sb[:, c * NPG:(c + 1) * NPG],
                                 ech[:], start=True, stop=True)
                # exp(lrelu(x)) == max(exp(x), exp(0.2x))
                e1 = ltpool.tile([NPG, CH], F16, tag="e1")
                nc.scalar.activation(e1[:], pl[:], AF.Exp)
                e2 = ltpool.tile([NPG, CH], F16, tag="e2")
                nc.scalar.activation(e2[:], pl[:], AF.Exp, scale=NEG_SLOPE)
                nc.vector.tensor_tensor(ex[:, cs], e1[:], e2[:], ALU.max)
            return pT, ex

        hT_in = x_sb
        for l in range(2):
            pT, ex = projection_and_logits(l, hT_in, 67)
            # node-major [adst | h~ | 1] blocks via per-graph PE transposes
            hnode = npool.tile([NPG, GPC * 66], F16, tag="hnode")
            for gs in _graph_banks(GPC, 7):
                pt = pst.tile([NPG, 66 * len(gs)], F16, tag="pt")
                for j, g in enumerate(gs):
                    nc.tensor.transpose(pt[:, j * 66:(j + 1) * 66],
                                        pT[:66, g * NPG:(g + 1) * NPG],
                                        ident_sb[:66, :66])
                nc.vector.tensor_copy(hnode[:, gs[0] * 66:(gs[-1] + 1) * 66], pt[:])
            # per-graph aggregation: [agg(64) | den] in one matmul
            recip = smpool.tile([NPG, GPC], F32, tag="recip")
            UN = upool.tile([NPG, GPC * HID], F16, tag="UN")
            for gs in _graph_banks(GPC, 7):
                pa = psa.tile([NPG, 65 * len(gs)], F32, tag="pa")
                for j, g in enumerate(gs):
                    exg = ex[:, g * NPG:(g + 1) * NPG]
                    o0 = j * 65
                    nc.tensor.matmul(pa[:, o0:o0 + 65], exg,
                                     hnode[:, g * 66 + 1:g * 66 + 66],
                                     start=True, stop=not use_bias[l])
                    if use_bias[l]:
                        nc.tensor.matmul(pa[:, o0:o0 + 64], exg, bb_sb[l][:],
                                         start=False, stop=True)
                gsl = slice(gs[0], gs[-1] + 1)
                nc.vector.reciprocal(recip[:, gsl], pa[:, 64::65])
                # UN = max(agg, 0) * (1/den), fused (stride-0 bcast)
                pa3 = pa[:].rearrange("p (g c) -> p g c", c=65)[:, :, 0:64]
                un3 = (UN[:, gs[0] * HID:(gs[-1] + 1) * HID]
                       .rearrange("p (g c) -> p g c", c=64))
                rb = _bcast_inner(recip[:, gsl], 64)
                nc.vector.scalar_tensor_tensor(un3, pa3, 0.0, rb,
                                               ALU.max, ALU.mult)
            # transpose pairs back to feature-major [65, NB] (row 64 = ones)
            hT_next = hpool.tile([HID + 1, NB], F16, tag="hT")
            nc.gpsimd.memset(hT_next[HID:HID + 1, :], 1.0)
            pair_banks = _graph_banks(GPC // 2, 6)   # 16 pairs, banks of 6
            for pb in pair_banks:
                ntr = len(pb)
                pt2 = pst2.tile([128, NPG * ntr], F16, tag="pt2")
                for t, pj in enumerate(pb):
                    nc.tensor.transpose(
                        pt2[:, t * NPG:(t + 1) * NPG],
                        UN[:, (2 * pj) * HID:(2 * pj + 2) * HID],
                        ident_sb[:])
                g0 = 2 * pb[0]
                dst = (hT_next[0:HID, :]
                       .rearrange("p (g s) -> p g s", s=NPG))
                src = pt2[:].rearrange("p (t s) -> p t s", s=NPG)
                nc.scalar.copy(dst[:, g0:g0 + 2 * ntr:2, :], src[0:HID])
                nc.vector.tensor_copy(dst[:, g0 + 1:g0 + 2 * ntr:2, :],
                                      src[HID:2 * HID])
            hT_in = hT_next

        # ---- layer 2 (readout folded in) ----
        pT, ex = projection_and_logits(2, hT_in, 68)
        nc.sync.dma_start(v_tmp.rearrange("(o n) -> o n", o=1),
                          pT[C_V:C_V + 1, :])
        v_mat = smpool.tile([NPG, GPC], F16, tag="vmat")
        nc.sync.dma_start(v_mat[:], v_tmp.rearrange("(g s) -> s g", g=GPC))
        # interleaved [v | 1] columns so each graph aggregates with one matmul
        vo = smpool.tile([NPG, 2 * GPC], F16, tag="vo")
        nc.vector.memset(vo[:], 1.0)
        nc.vector.tensor_copy(vo[:, 0::2], v_mat[:])

        pq = psa.tile([NPG, 2 * GPC], F32, tag="pa")
        for g in range(GPC):
            nc.tensor.matmul(pq[:, 2 * g:2 * g + 2],
                             ex[:, g * NPG:(g + 1) * NPG],
                             vo[:, 2 * g:2 * g + 2], start=True, stop=True)
        recip2 = smpool.tile([NPG, GPC], F32, tag="recip")
        nc.vector.reciprocal(recip2[:], pq[:, 1::2])
        qsb = smpool.tile([NPG, GPC], F32, tag="qsb")
        nc.vector.tensor_mul(qsb[:], pq[:, 0::2], recip2[:])
        # exact fp32 pooling: bounce [d, g] -> [g, d], then free-axis reduce
        nc.sync.dma_start(q_tmp.rearrange("(s g) -> s g", g=GPC), qsb[:])
        qT = smpool.tile([GPC, NPG], F32, tag="qT")
        nc.sync.dma_start(qT[:], q_tmp.rearrange("(s g) -> g s", g=GPC))
        zcol = smpool.tile([GPC, 1], F32, tag="zcol")
        nc.vector.reduce_sum(zcol[:], qT[:], axis=mybir.AxisListType.X)
        zout = smpool.tile([GPC, 1], F32, tag="zout")
        nc.scalar.activation(zout[:], zcol[:], AF.Relu, bias=float(tail_bias))
        nc.sync.dma_start(out_d.rearrange("(g o) -> g o", o=1), zout[:])

    nc.compile()
    return nc


def _core_inputs(pre, c):
    m = {
        'xT': np.ascontiguousarray(pre['x_aug'][:, c * NB:(c + 1) * NB]),
        'maskc': pre['maskc'], 'lhs92': pre['lhs92'], 'ident': pre['ident'],
    }
    for l in range(DEPTH):
        m[f'E{l}'] = np.ascontiguousarray(
            np.transpose(pre['Es'][l][c * GPC:(c + 1) * GPC], (1, 0, 2))
            .reshape(NPG, NB))
        m[f'CW{l}'] = pre['CW'][l]
    for l in range(2):
        if np.any(pre['bl'][l] != 0):
            m[f'bb{l}'] = np.ascontiguousarray(
                np.tile(pre['bl'][l][None, :], (NPG, 1)).astype(np.float16))
    return m


def kernel(**inputs):
    pre = _host_preprocess(inputs)
    use_bias = tuple(bool(np.any(pre['bl'][l] != 0)) for l in range(2))
    nc = _build_program(pre['tail_bias'], use_bias)
    in_maps = [_core_inputs(pre, c) for c in range(NC_CORES)]
    res = run_bass_kernel_spmd(nc, in_maps, list(range(NC_CORES)))
    out = np.concatenate([np.asarray(res.results[c]['out'])
                          for c in range(NC_CORES)])
    return out.reshape(B, 1).astype(np.float32)



# revision 13
# speedup vs baseline: 1.2432x; 1.2432x over previous
"""Trainium2 Bass kernel for a 3-layer edge-featured GAT over 256 dense 84-node graphs.

Contract: kernel(**inputs) takes the FULL unsharded inputs (as produced by the
problem's setup_inputs) and returns the FULL [256, 1] float32 output.

Strategy (data parallel over graphs, 32 graphs/core on 8 cores):
  Each graph is dense (all ordered pairs + self loops), so message passing
  collapses to dense per-graph [84, 84] attention matrices. Host-side we
  scatter edge_attr into dense per-graph planes (folding the per-layer edge
  MLP down to a scalar per edge, and the PyG mean self-loop attr onto the
  diagonal), fold a_src/a_dst/readout into augmented layer weights, and keep
  a constant-one input feature so every projection carries a ones column
  (which turns the softmax denominator into one extra matmul column).

  Per layer on device: one combined projection produces, per node, the
  projected features h~, a_src/a_dst attention scalars and a constant 1;
  the [src, dst] logit plane is produced by one matmul per 448-col chunk
  whose moving operand is a persistent [92, NB] tile (rows 0:84 = the host
  E plane, DMA'd once per layer; rows 84:91 = block masks, loaded once;
  row 91 = the runtime a_dst row, written by a cheap DVE copy) against a
  stationary that carries identity / runtime a_src rows / ones;
  exp(lrelu(x)) is computed as max(exp(x), exp(0.2 x)); per-graph matmuls
  of ex_g against node-major [h~ | 1] give aggregate + denominator in one
  pass; relu and the 1/den normalization fuse into one strided
  scalar_tensor_tensor per PSUM bank with a stride-0 broadcast AP.

  The final global_add_pool is a single fp32 ones-column matmul over the
  per-node readout (no DRAM transpose bounce), followed by relu+bias.

  All matmul operands are fp16 (PSUM accumulation stays fp32) except the
  tiny exact pooling matmul.
"""

import sys

for _p in ("/opt/trn_rl_repo",):
    if _p not in sys.path:
        sys.path.append(_p)

import numpy as np

from contextlib import ExitStack

from concourse import bacc, bass, mybir, tile
from concourse.bass_types import AP
from concourse.bass_utils import run_bass_kernel_spmd

F32 = mybir.dt.float32
F16 = mybir.dt.float16
AF = mybir.ActivationFunctionType
ALU = mybir.AluOpType

NPG = 84            # nodes per graph
B = 256             # graphs
HID = 64
DEPTH = 3
NEG_SLOPE = 0.2
NC_CORES = 8
GPC = B // NC_CORES     # 32 graphs per core
NB = GPC * NPG          # 2688 nodes per core
CH = 448                # free-dim chunk (one PSUM bank)
NCH = NB // CH          # 6 chunks

# projection column layout: [a_dst | W(64) | ones | a_src | v(layer2)]
C_ADST, C_W0, C_ONE, C_ASRC, C_V = 0, 1, 65, 66, 67


def _chunk_graphs(c):
    """Graphs whose columns intersect chunk c."""
    g_lo = (CH * c) // NPG
    g_hi = (CH * (c + 1) - 1) // NPG
    return g_lo, min(g_hi, GPC - 1)


def _host_preprocess(inputs):
    x = np.ascontiguousarray(np.asarray(inputs['x'], np.float32))
    ei = np.asarray(inputs['edge_index'])
    ea = np.asarray(inputs['edge_attr'], np.float32)
    W0 = np.asarray(inputs['W0'], np.float32)
    Ws = np.asarray(inputs['Ws'], np.float32)
    asl = np.asarray(inputs['att_src_all'], np.float32)
    adl = np.asarray(inputs['att_dst_all'], np.float32)
    Wel = np.asarray(inputs['W_edge_all'], np.float32)
    ael = np.asarray(inputs['att_edge_all'], np.float32)
    bl = np.asarray(inputs['bias_all'], np.float32)
    linW = np.asarray(inputs['lin_W'], np.float32)
    linb = np.asarray(inputs['lin_b'], np.float32)

    src, dst = np.asarray(ei[0]), np.asarray(ei[1])
    g = src // NPG
    assert np.all(dst // NPG == g), "edges cross graph boundaries"
    sl, dl = src % NPG, dst % NPG

    dense = np.zeros((B, NPG, NPG, 2), np.float32)
    dense[g, sl, dl] = ea
    cnt = np.zeros((B, NPG), np.float32)
    np.add.at(cnt, (g, dl), 1.0)
    colsum = dense.sum(axis=1)
    loop_attr = colsum / np.maximum(cnt, 1.0)[..., None]
    di = np.arange(NPG)
    dense[:, di, di, :] = loop_attr

    Es = []
    for l in range(DEPTH):
        w2 = Wel[l] @ ael[l]
        Es.append(np.ascontiguousarray(dense @ w2, dtype=np.float16))

    W_all = [W0, Ws[0], Ws[1]]
    CW = []
    for l in range(DEPTH):
        K = W_all[l].shape[0]
        cols = [(W_all[l] @ adl[l])[:, None], W_all[l], np.zeros((K, 1), np.float32),
                (W_all[l] @ asl[l])[:, None]]
        if l == DEPTH - 1:
            cols.append(W_all[l] @ linW)
        A = np.concatenate(cols, axis=1)
        aug = np.zeros((1, A.shape[1]), np.float32)
        aug[0, C_ONE] = 1.0
        CW.append(np.ascontiguousarray(np.vstack([A, aug]), np.float16))

    tail_bias = float(NPG * float(bl[DEPTH - 1] @ linW[:, 0]) + float(linb[0]))

    # per-chunk block-diagonal masks: row k of chunk c covers graph g_lo(c)+k
    maskc = np.zeros((7, NB), np.float16)
    for c in range(NCH):
        g_lo, _ = _chunk_graphs(c)
        for j in range(CH):
            gg = (CH * c + j) // NPG
            maskc[gg - g_lo, CH * c + j] = 1.0
    # merged-logits stationary: row 0 ones (pairs with the runtime a_dst row,
    # which must sit at partition 0 for the DVE row copy), rows 1..84 identity
    # (pairs with E), rows 85..91 runtime a_src rows (pair with maskc)
    lhs92 = np.zeros((92, NCH * NPG), np.float16)
    lhs92[0, :] = 1.0
    for c in range(NCH):
        lhs92[1:NPG + 1, c * NPG:(c + 1) * NPG] = np.eye(NPG, dtype=np.float16)
    ident = np.eye(NPG, dtype=np.float16)
    x_aug = np.ones((2, B * NPG), np.float16)
    x_aug[0] = x[:, 0].astype(np.float16)

    return dict(x_aug=x_aug, Es=Es, CW=CW, bl=bl, tail_bias=tail_bias,
                maskc=maskc, lhs92=lhs92, ident=ident)


def _graph_banks(n_graphs, per_bank):
    out = []
    g0 = 0
    while g0 < n_graphs:
        out.append(list(range(g0, min(g0 + per_bank, n_graphs))))
        g0 += per_bank
    return out


def _bcast_inner(ap, n):
    """View `ap` with an extra innermost stride-0 axis of length n."""
    return AP(ap.tensor, ap.offset, list(ap.ap) + [[0, n]])


def _build_program(tail_bias, use_bias):
    """use_bias: (bool, bool) for layers 0 and 1 (per-node bias via ex@bb matmul)."""
    nc = bacc.Bacc("TRN2", target_bir_lowering=False, debug=False)

    xT_d = nc.dram_tensor("xT", [2, NB], F16, kind="ExternalInput").ap()
    E_d = [nc.dram_tensor(f"E{l}", [NPG, NB], F16, kind="ExternalInput").ap()
           for l in range(DEPTH)]
    ncw = [67, 67, 68]
    CW_d = [nc.dram_tensor(f"CW{l}", [(2 if l == 0 else HID + 1), ncw[l]],
                           F16, kind="ExternalInput").ap() for l in range(DEPTH)]
    maskc_d = nc.dram_tensor("maskc", [7, NB], F16, kind="ExternalInput").ap()
    lhs92_d = nc.dram_tensor("lhs92", [92, NCH * NPG], F16, kind="ExternalInput").ap()
    ident_d = nc.dram_tensor("ident", [NPG, NPG], F16, kind="ExternalInput").ap()
    bb_d = [nc.dram_tensor(f"bb{l}", [NPG, HID], F16, kind="ExternalInput").ap()
            if use_bias[l] else None for l in range(2)]
    # row bounce scratch (sbuf row -> dram -> repartitioned sbuf)
    asrc_tmp = [nc.dram_tensor(f"asrc_tmp{l}", [NB], F16).ap() for l in range(2)]
    av_tmp = nc.dram_tensor("av_tmp", [2, NB], F16).ap()   # layer2: [asrc; v]
    out_d = nc.dram_tensor("out", [GPC], F32, kind="ExternalOutput").ap()

    with tile.TileContext(nc) as tc, ExitStack() as ctx:
        cpool = ctx.enter_context(tc.tile_pool(name="const", bufs=1))
        hpool = ctx.enter_context(tc.tile_pool(name="h", bufs=2))
        ppool = ctx.enter_context(tc.tile_pool(name="proj", bufs=2))
        npool = ctx.enter_context(tc.tile_pool(name="hnode", bufs=2))
        expool = ctx.enter_context(tc.tile_pool(name="ex", bufs=2))
        ltpool = ctx.enter_context(tc.tile_pool(name="lt", bufs=3))
        smpool = ctx.enter_context(tc.tile_pool(name="small", bufs=3))
        upool = ctx.enter_context(tc.tile_pool(name="u", bufs=2))

        psb = ctx.enter_context(tc.tile_pool(name="psb", bufs=3, space="PSUM"))
        psa = ctx.enter_context(tc.tile_pool(name="psa", bufs=2, space="PSUM"))
        pst = ctx.enter_context(tc.tile_pool(name="pst", bufs=1, space="PSUM"))
        pst2 = ctx.enter_context(tc.tile_pool(name="pst2", bufs=1, space="PSUM"))
        pszp = ctx.enter_context(tc.tile_pool(name="pszp", bufs=1, space="PSUM"))

        # constants — small critical inputs (x, CW, ident, lhs92) first so the
        # first projection/transpose work isn't queued behind the E planes
        x_sb = hpool.tile([2, NB], F16, tag="x")
        nc.sync.dma_start(x_sb[:], xT_d[:])
        cw_sb = []
        for l in range(DEPTH):
            t = cpool.tile(list(CW_d[l].shape), F16, tag=f"cw{l}")
            nc.sync.dma_start(t[:], CW_d[l][:])
            cw_sb.append(t)
        ident_sb = cpool.tile([NPG, NPG], F16, tag="ident")
        nc.sync.dma_start(ident_sb[:], ident_d[:])
        lhs92_sb = cpool.tile([92, NCH * NPG], F16, tag="lhs92")
        nc.sync.dma_start(lhs92_sb[:], lhs92_d[:])
        bb_sb = []
        for l in range(2):
            if use_bias[l]:
                t = cpool.tile([NPG, HID], F16, tag=f"bb{l}")
                nc.sync.dma_start(t[:], bb_d[l][:])
                bb_sb.append(t)
            else:
                bb_sb.append(None)

        # persistent logits moving-operand tiles: row 0 = runtime a_dst (per
        # chunk), rows 1:85 = E (per layer), rows 85:92 = maskc (once)
        lsrc = [cpool.tile([92, NB], F16, tag=f"lsrc{i}", name=f"lsrc{i}")
                for i in range(2)]
        for i in range(2):
            nc.gpsimd.dma_start(lsrc[i][NPG + 1:NPG + 8, :], maskc_d[:])
            nc.sync.dma_start(lsrc[i][1:NPG + 1, :], E_d[i][:])

        # layer-2 interleaved [v | 1] aggregation operand; ones set up front
        vo = smpool.tile([NPG, 2 * GPC], F16, tag="vo")
        nc.gpsimd.memset(vo[:], 1.0)
        # fp32 ones column for the exact pooling matmul
        ones84 = smpool.tile([NPG, 1], F32, tag="ones84")
        nc.vector.memset(ones84[:], 1.0)

        def projection(l, hT_in, nrows):
            """Projection + a_src/a_dst row distribution. Returns pT."""
            ls = lsrc[l % 2]
            pT = ppool.tile([nrows, NB], F16, tag="pT")
            for c in range(NCH):
                cs = slice(c * CH, (c + 1) * CH)
                pw = psb.tile([nrows, CH], F32, tag="pb")
                nc.tensor.matmul(pw[:], cw_sb[l][:], hT_in[:, cs],
                                 start=True, stop=True)
                if c % 2 == 0:
                    nc.scalar.copy(pT[:, cs], pw[:])
                else:
                    nc.vector.tensor_copy(pT[:, cs], pw[:])
                # a_dst row into the persistent logits tile (cheap DVE row copy)
                nc.vector.tensor_copy(ls[0:1, cs], pT[C_ADST:C_ADST + 1, cs])
                # bounce this chunk's a_src row immediately (plus v for layer 2)
                if l < 2:
                    nc.sync.dma_start(
                        asrc_tmp[l][cs].rearrange("(o n) -> o n", o=1),
                        pT[C_ASRC:C_ASRC + 1, cs])
                else:
                    nc.sync.dma_start(av_tmp[:, cs], pT[C_ASRC:C_V + 1, cs])
            for c in range(NCH):
                g_lo, g_hi = _chunk_graphs(c)
                ng = g_hi - g_lo + 1
                arow = asrc_tmp[l] if l < 2 else av_tmp[0]
                nc.sync.dma_start(
                    lhs92_sb[85:85 + ng, c * NPG:(c + 1) * NPG],
                    arow[g_lo * NPG:(g_hi + 1) * NPG]
                    .rearrange("(g s) -> g s", g=ng))
                if l == 2:
                    # v values for these graphs -> interleaved vo columns
                    nc.gpsimd.dma_start(
                        vo[:, 2 * g_lo:2 * (g_hi + 1):2],
                        av_tmp[1, g_lo * NPG:(g_hi + 1) * NPG]
                        .rearrange("(g s) -> s g", g=ng))
            return pT

        def logits(l):
            """Attention ex from the persistent logits tile + lhs92."""
            ls = lsrc[l % 2]
            ex = expool.tile([NPG, NB], F16, tag="ex")
            for c in range(NCH):
                cs = slice(c * CH, (c + 1) * CH)
                pl = psb.tile([NPG, CH], F32, tag="pb")
                nc.tensor.matmul(pl[:], lhs92_sb[:, c * NPG:(c + 1) * NPG],
                                 ls[:, cs], start=True, stop=True)
                # exp(lrelu(x)) == max(exp(x), exp(0.2x))
                e1 = ltpool.tile([NPG, CH], F16, tag="e1")
                nc.scalar.activation(e1[:], pl[:], AF.Exp)
                e2 = ltpool.tile([NPG, CH], F16, tag="e2")
                nc.scalar.activation(e2[:], pl[:], AF.Exp, scale=NEG_SLOPE)
                nc.vector.tensor_tensor(ex[:, cs], e1[:], e2[:], ALU.max)
            return ex

        hT_in = x_sb
        for l in range(2):
            pT = projection(l, hT_in, 67)
            # node-major [adst | h~ | 1] blocks via per-graph PE transposes.
            # Issued before the logits matmuls so the PE queue has work while
            # the a_src DRAM bounce completes.
            hnode = npool.tile([NPG, GPC * 66], F16, tag="hnode")
            for gs in _graph_banks(GPC, 7):
                pt = pst.tile([NPG, 66 * len(gs)], F16, tag="pt")
                for j, g in enumerate(gs):
                    nc.tensor.transpose(pt[:, j * 66:(j + 1) * 66],
                                        pT[:66, g * NPG:(g + 1) * NPG],
                                        ident_sb[:66, :66])
                nc.vector.tensor_copy(hnode[:, gs[0] * 66:(gs[-1] + 1) * 66], pt[:])
            ex = logits(l)
            # per-graph aggregation: [agg(64) | den] in one matmul
            recip = smpool.tile([NPG, GPC], F32, tag="recip")
            UN = upool.tile([NPG, GPC * HID], F16, tag="UN")
            for gs in _graph_banks(GPC, 7):
                pa = psa.tile([NPG, 65 * len(gs)], F32, tag="pa")
                for j, g in enumerate(gs):
                    exg = ex[:, g * NPG:(g + 1) * NPG]
                    o0 = j * 65
                    nc.tensor.matmul(pa[:, o0:o0 + 65], exg,
                                     hnode[:, g * 66 + 1:g * 66 + 66],
                                     start=True, stop=not use_bias[l])
                    if use_bias[l]:
                        nc.tensor.matmul(pa[:, o0:o0 + 64], exg, bb_sb[l][:],
                                         start=False, stop=True)
                gsl = slice(gs[0], gs[-1] + 1)
                nc.vector.reciprocal(recip[:, gsl], pa[:, 64::65])
                # UN = max(agg, 0) * (1/den), fused (stride-0 bcast)
                pa3 = pa[:].rearrange("p (g c) -> p g c", c=65)[:, :, 0:64]
                un3 = (UN[:, gs[0] * HID:(gs[-1] + 1) * HID]
                       .rearrange("p (g c) -> p g c", c=64))
                rb = _bcast_inner(recip[:, gsl], 64)
                nc.vector.scalar_tensor_tensor(un3, pa3, 0.0, rb,
                                               ALU.max, ALU.mult)
            # transpose pairs back to feature-major [65, NB] (row 64 = ones)
            hT_next = hpool.tile([HID + 1, NB], F16, tag="hT")
            nc.gpsimd.memset(hT_next[HID:HID + 1, :], 1.0)
            pair_banks = _graph_banks(GPC // 2, 6)   # 16 pairs, banks of 6
            for pb in pair_banks:
                ntr = len(pb)
                pt2 = pst2.tile([128, NPG * ntr], F16, tag="pt2")
                for t, pj in enumerate(pb):
                    nc.tensor.transpose(
                        pt2[:, t * NPG:(t + 1) * NPG],
                        UN[:, (2 * pj) * HID:(2 * pj + 2) * HID],
                        ident_sb[:])
                g0 = 2 * pb[0]
                dst = (hT_next[0:HID, :]
                       .rearrange("p (g s) -> p g s", s=NPG))
                src = pt2[:].rearrange("p (t s) -> p t s", s=NPG)
                nc.scalar.copy(dst[:, g0:g0 + 2 * ntr:2, :], src[0:HID])
                nc.vector.tensor_copy(dst[:, g0 + 1:g0 + 2 * ntr:2, :],
                                      src[HID:2 * HID])
            hT_in = hT_next
            if l == 0:
                # layer 2's E plane replaces layer 0's (same buffer), WAR-safe
                nc.sync.dma_start(lsrc[0][1:NPG + 1, :], E_d[2][:])

        # ---- layer 2 (readout folded in) ----
        pT = projection(2, hT_in, 68)
        ex = logits(2)

        pq = psa.tile([NPG, 2 * GPC], F32, tag="pa")
        for g in range(GPC):
            nc.tensor.matmul(pq[:, 2 * g:2 * g + 2],
                             ex[:, g * NPG:(g + 1) * NPG],
                             vo[:, 2 * g:2 * g + 2], start=True, stop=True)
        recip2 = smpool.tile([NPG, GPC], F32, tag="recip")
        nc.vector.reciprocal(recip2[:], pq[:, 1::2])
        qsb = smpool.tile([NPG, GPC], F32, tag="qsb")
        nc.vector.tensor_mul(qsb[:], pq[:, 0::2], recip2[:])
        # exact fp32 pooling: one ones-column matmul sums the partition axis
        zp = pszp.tile([1, GPC], F32, tag="zp")
        nc.tensor.matmul(zp[:], ones84[:], qsb[:], start=True, stop=True)
        zout = smpool.tile([1, GPC], F32, tag="zout")
        nc.scalar.activation(zout[:], zp[:], AF.Relu, bias=float(tail_bias))
        nc.sync.dma_start(out_d.rearrange("(o g) -> o g", o=1), zout[:])

    nc.compile()
    return nc


def _core_inputs(pre, c):
    m = {
        'xT': np.ascontiguousarray(pre['x_aug'][:, c * NB:(c + 1) * NB]),
        'maskc': pre['maskc'], 'lhs92': pre['lhs92'], 'ident': pre['ident'],
    }
    for l in range(DEPTH):
        m[f'E{l}'] = np.ascontiguousarray(
            np.transpose(pre['Es'][l][c * GPC:(c + 1) * GPC], (1, 0, 2))
            .reshape(NPG, NB))
        m[f'CW{l}'] = pre['CW'][l]
    for l in range(2):
        if np.any(pre['bl'][l] != 0):
            m[f'bb{l}'] = np.ascontiguousarray(
                np.tile(pre['bl'][l][None, :], (NPG, 1)).astype(np.float16))
    return m


def kernel(**inputs):
    pre = _host_preprocess(inputs)
    use_bias = tuple(bool(np.any(pre['bl'][l] != 0)) for l in range(2))
    nc = _build_program(pre['tail_bias'], use_bias)
    in_maps = [_core_inputs(pre, c) for c in range(NC_CORES)]
    res = run_bass_kernel_spmd(nc, in_maps, list(range(NC_CORES)))
    out = np.concatenate([np.asarray(res.results[c]['out'])
                          for c in range(NC_CORES)])
    return out.reshape(B, 1).astype(np.float32)


# revision 18
# speedup vs baseline: 1.3675x; 1.1000x over previous
"""Trainium2 Bass kernel for a 3-layer edge-featured GAT over 256 dense 84-node graphs.

Contract: kernel(**inputs) takes the FULL unsharded inputs (as produced by the
problem's setup_inputs) and returns the FULL [256, 1] float32 output.

Strategy (data parallel over graphs, 32 graphs/core on 8 cores):
  Each graph is dense (all ordered pairs + self loops), so message passing
  collapses to dense per-graph [84, 84] attention matrices. Host-side we
  scatter edge_attr into dense per-graph planes (folding the per-layer edge
  MLP down to a scalar per edge, and the PyG mean self-loop attr onto the
  diagonal), fold a_src/a_dst/readout into augmented layer weights, and keep
  a constant-one input feature so every projection carries a ones column
  (which turns the softmax denominator into one extra matmul column).

  Per layer on device: one combined projection produces, per node, the
  projected features h~, a_src/a_dst attention scalars and a constant 1;
  the [src, dst] logit plane is produced by one matmul per 448-col chunk
  whose moving operand is a persistent [92, NB] tile (rows 0:84 = the host
  E plane, DMA'd once per layer; rows 84:91 = block masks, loaded once;
  row 91 = the runtime a_dst row, written by a cheap DVE copy) against a
  stationary that carries identity / runtime a_src rows / ones;
  exp(lrelu(x)) is computed as max(exp(x), exp(0.2 x)); per-graph matmuls
  of ex_g against node-major [h~ | 1] give aggregate + denominator in one
  pass; relu and the 1/den normalization fuse into one strided
  scalar_tensor_tensor per PSUM bank with a stride-0 broadcast AP.

  The final global_add_pool is a single fp32 ones-column matmul over the
  per-node readout (no DRAM transpose bounce), followed by relu+bias.

  All matmul operands are fp16 (PSUM accumulation stays fp32) except the
  tiny exact pooling matmul.
"""

import sys

for _p in ("/opt/trn_rl_repo",):
    if _p not in sys.path:
        sys.path.append(_p)

import numpy as np

from contextlib import ExitStack

from concourse import bacc, bass, mybir, tile
from concourse.bass_types import AP
from concourse.bass_utils import run_bass_kernel_spmd

F32 = mybir.dt.float32
F16 = mybir.dt.float16
AF = mybir.ActivationFunctionType
ALU = mybir.AluOpType

NPG = 84            # nodes per graph
B = 256             # graphs
HID = 64
DEPTH = 3
NEG_SLOPE = 0.2
NC_CORES = 8
GPC = B // NC_CORES     # 32 graphs per core
NB = GPC * NPG          # 2688 nodes per core
CH = 448                # free-dim chunk (one PSUM bank)
NCH = NB // CH          # 6 chunks

# projection column layout: [a_dst | W(64) | ones | a_src | v(layer2)]
C_ADST, C_W0, C_ONE, C_ASRC, C_V = 0, 1, 65, 66, 67


def _chunk_graphs(c):
    """Graphs whose columns intersect chunk c."""
    g_lo = (CH * c) // NPG
    g_hi = (CH * (c + 1) - 1) // NPG
    return g_lo, min(g_hi, GPC - 1)


def _host_preprocess(inputs):
    x = np.ascontiguousarray(np.asarray(inputs['x'], np.float32))
    ei = np.asarray(inputs['edge_index'])
    ea = np.asarray(inputs['edge_attr'], np.float32)
    W0 = np.asarray(inputs['W0'], np.float32)
    Ws = np.asarray(inputs['Ws'], np.float32)
    asl = np.asarray(inputs['att_src_all'], np.float32)
    adl = np.asarray(inputs['att_dst_all'], np.float32)
    Wel = np.asarray(inputs['W_edge_all'], np.float32)
    ael = np.asarray(inputs['att_edge_all'], np.float32)
    bl = np.asarray(inputs['bias_all'], np.float32)
    linW = np.asarray(inputs['lin_W'], np.float32)
    linb = np.asarray(inputs['lin_b'], np.float32)

    src, dst = np.asarray(ei[0]), np.asarray(ei[1])
    g = src // NPG
    assert np.all(dst // NPG == g), "edges cross graph boundaries"
    sl, dl = src % NPG, dst % NPG

    dense = np.zeros((B, NPG, NPG, 2), np.float32)
    dense[g, sl, dl] = ea
    cnt = np.zeros((B, NPG), np.float32)
    np.add.at(cnt, (g, dl), 1.0)
    colsum = dense.sum(axis=1)
    loop_attr = colsum / np.maximum(cnt, 1.0)[..., None]
    di = np.arange(NPG)
    dense[:, di, di, :] = loop_attr

    Es = []
    for l in range(DEPTH):
        w2 = Wel[l] @ ael[l]
        Es.append(np.ascontiguousarray(dense @ w2, dtype=np.float16))
    # layer 0's attention rows depend only on the (known) input x — fold them
    # into the layer-0 E plane so no logits matmul / a_src bounce is needed
    asrc0 = (x[:, 0] * float(W0[0] @ asl[0])).reshape(B, NPG)
    adst0 = (x[:, 0] * float(W0[0] @ adl[0])).reshape(B, NPG)
    Es[0] = np.ascontiguousarray(
        (dense @ (Wel[0] @ ael[0])).astype(np.float32)
        + asrc0[:, :, None] + adst0[:, None, :], np.float16)

    W_all = [W0, Ws[0], Ws[1]]
    CW = []
    for l in range(DEPTH):
        K = W_all[l].shape[0]
        cols = [(W_all[l] @ adl[l])[:, None], W_all[l], np.zeros((K, 1), np.float32),
                (W_all[l] @ asl[l])[:, None]]
        if l == DEPTH - 1:
            cols.append(W_all[l] @ linW)
        A = np.concatenate(cols, axis=1)
        aug = np.zeros((1, A.shape[1]), np.float32)
        aug[0, C_ONE] = 1.0
        CW.append(np.ascontiguousarray(np.vstack([A, aug]), np.float16))

    tail_bias = float(NPG * float(bl[DEPTH - 1] @ linW[:, 0]) + float(linb[0]))

    # per-chunk block-diagonal masks: row k of chunk c covers graph g_lo(c)+k
    maskc = np.zeros((7, NB), np.float16)
    for c in range(NCH):
        g_lo, _ = _chunk_graphs(c)
        for j in range(CH):
            gg = (CH * c + j) // NPG
            maskc[gg - g_lo, CH * c + j] = 1.0
    # merged-logits stationary: row 0 ones (pairs with the runtime a_dst row,
    # which must sit at partition 0 for the DVE row copy), rows 1..84 identity
    # (pairs with E), rows 85..91 runtime a_src rows (pair with maskc)
    lhs92 = np.zeros((92, NCH * NPG), np.float16)
    lhs92[0, :] = 1.0
    for c in range(NCH):
        lhs92[1:NPG + 1, c * NPG:(c + 1) * NPG] = np.eye(NPG, dtype=np.float16)
    ident = np.eye(NPG, dtype=np.float16)
    x_aug = np.ones((2, B * NPG), np.float16)
    x_aug[0] = x[:, 0].astype(np.float16)

    return dict(x_aug=x_aug, Es=Es, CW=CW, bl=bl, tail_bias=tail_bias,
                maskc=maskc, lhs92=lhs92, ident=ident)


def _graph_banks(n_graphs, per_bank):
    out = []
    g0 = 0
    while g0 < n_graphs:
        out.append(list(range(g0, min(g0 + per_bank, n_graphs))))
        g0 += per_bank
    return out


def _bcast_inner(ap, n):
    """View `ap` with an extra innermost stride-0 axis of length n."""
    return AP(ap.tensor, ap.offset, list(ap.ap) + [[0, n]])


def _build_program(tail_bias, use_bias):
    """use_bias: (bool, bool) for layers 0 and 1 (per-node bias via ex@bb matmul)."""
    nc = bacc.Bacc("TRN2", target_bir_lowering=False, debug=False)

    xT_d = nc.dram_tensor("xT", [2, NB], F16, kind="ExternalInput").ap()
    E_d = [nc.dram_tensor(f"E{l}", [NPG, NB], F16, kind="ExternalInput").ap()
           for l in range(DEPTH)]
    ncw = [67, 67, 68]
    CW_d = [nc.dram_tensor(f"CW{l}", [(2 if l == 0 else HID + 1), ncw[l]],
                           F16, kind="ExternalInput").ap() for l in range(DEPTH)]
    maskc_d = nc.dram_tensor("maskc", [7, NB], F16, kind="ExternalInput").ap()
    lhs92_d = nc.dram_tensor("lhs92", [92, NCH * NPG], F16, kind="ExternalInput").ap()
    ident_d = nc.dram_tensor("ident", [NPG, NPG], F16, kind="ExternalInput").ap()
    bb_d = [nc.dram_tensor(f"bb{l}", [NPG, HID], F16, kind="ExternalInput").ap()
            if use_bias[l] else None for l in range(2)]
    # row bounce scratch (sbuf row -> dram -> repartitioned sbuf)
    asrc_tmp = [nc.dram_tensor(f"asrc_tmp{l}", [NB], F16).ap() for l in range(2)]
    av_tmp = nc.dram_tensor("av_tmp", [2, NB], F16).ap()   # layer2: [asrc; v]
    out_d = nc.dram_tensor("out", [GPC], F32, kind="ExternalOutput").ap()

    with tile.TileContext(nc) as tc, ExitStack() as ctx:
        cpool = ctx.enter_context(tc.tile_pool(name="const", bufs=1))
        hpool = ctx.enter_context(tc.tile_pool(name="h", bufs=2))
        ppool = ctx.enter_context(tc.tile_pool(name="proj", bufs=2))
        npool = ctx.enter_context(tc.tile_pool(name="hnode", bufs=2))
        expool = ctx.enter_context(tc.tile_pool(name="ex", bufs=2))
        ltpool = ctx.enter_context(tc.tile_pool(name="lt", bufs=3))
        smpool = ctx.enter_context(tc.tile_pool(name="small", bufs=3))
        upool = ctx.enter_context(tc.tile_pool(name="u", bufs=2))

        psb = ctx.enter_context(tc.tile_pool(name="psb", bufs=3, space="PSUM"))
        psa = ctx.enter_context(tc.tile_pool(name="psa", bufs=2, space="PSUM"))
        pst = ctx.enter_context(tc.tile_pool(name="pst", bufs=1, space="PSUM"))
        pst2 = ctx.enter_context(tc.tile_pool(name="pst2", bufs=1, space="PSUM"))
        pszp = ctx.enter_context(tc.tile_pool(name="pszp", bufs=1, space="PSUM"))

        # constants — small critical inputs (x, CW, ident, lhs92) first so the
        # first projection/transpose work isn't queued behind the E planes
        x_sb = hpool.tile([2, NB], F16, tag="x")
        nc.sync.dma_start(x_sb[:], xT_d[:])
        cw_sb = []
        for l in range(DEPTH):
            t = cpool.tile(list(CW_d[l].shape), F16, tag=f"cw{l}")
            nc.sync.dma_start(t[:], CW_d[l][:])
            cw_sb.append(t)
        ident_sb = cpool.tile([NPG, NPG], F16, tag="ident")
        nc.sync.dma_start(ident_sb[:], ident_d[:])
        lhs92_sb = cpool.tile([92, NCH * NPG], F16, tag="lhs92")
        nc.sync.dma_start(lhs92_sb[:], lhs92_d[:])
        bb_sb = []
        for l in range(2):
            if use_bias[l]:
                t = cpool.tile([NPG, HID], F16, tag=f"bb{l}")
                nc.sync.dma_start(t[:], bb_d[l][:])
                bb_sb.append(t)
            else:
                bb_sb.append(None)

        # layer 0: fully folded E plane (E + a_src + a_dst), exp'd directly
        E0sb = cpool.tile([NPG, NB], F16, tag="E0sb")
        nc.sync.dma_start(E0sb[:], E_d[0][:])
        # persistent logits moving-operand tiles for layers 1/2: row 0 =
        # runtime a_dst (per chunk), rows 1:85 = E, rows 85:92 = maskc (once)
        lsrc = [cpool.tile([92, NB], F16, tag=f"lsrc{i}", name=f"lsrc{i}")
                for i in range(2)]
        for i in range(2):
            nc.gpsimd.dma_start(lsrc[i][NPG + 1:NPG + 8, :], maskc_d[:])
            nc.sync.dma_start(lsrc[i][1:NPG + 1, :], E_d[2 - i][:])

        # layer-2 interleaved [v | 1] aggregation operand; ones set up front
        vo = smpool.tile([NPG, 2 * GPC], F16, tag="vo")
        nc.gpsimd.memset(vo[:], 1.0)
        # fp32 ones column for the exact pooling matmul
        ones84 = smpool.tile([NPG, 1], F32, tag="ones84")
        nc.vector.memset(ones84[:], 1.0)

        def projection(l, hT_in, nrows):
            """Projection + a_src/a_dst row distribution. Returns pT."""
            ls = lsrc[l % 2]
            pT = ppool.tile([nrows, NB], F16, tag="pT")
            for c in range(NCH):
                cs = slice(c * CH, (c + 1) * CH)
                pw = psb.tile([nrows, CH], F32, tag="pb")
                nc.tensor.matmul(pw[:], cw_sb[l][:], hT_in[:, cs],
                                 start=True, stop=True)
                if c % 2 == 0:
                    nc.scalar.copy(pT[:, cs], pw[:])
                else:
                    nc.vector.tensor_copy(pT[:, cs], pw[:])
                if l == 0:
                    continue    # layer 0 attention rows are host-folded
                # a_dst row into the persistent logits tile (cheap DVE row copy)
                nc.vector.tensor_copy(ls[0:1, cs], pT[C_ADST:C_ADST + 1, cs])
                # bounce this chunk's a_src row immediately (plus v for layer 2)
                if l < 2:
                    nc.sync.dma_start(
                        asrc_tmp[l][cs].rearrange("(o n) -> o n", o=1),
                        pT[C_ASRC:C_ASRC + 1, cs])
                else:
                    nc.sync.dma_start(av_tmp[:, cs], pT[C_ASRC:C_V + 1, cs])
            for c in range(NCH if l > 0 else 0):
                g_lo, g_hi = _chunk_graphs(c)
                ng = g_hi - g_lo + 1
                arow = asrc_tmp[l] if l < 2 else av_tmp[0]
                nc.sync.dma_start(
                    lhs92_sb[85:85 + ng, c * NPG:(c + 1) * NPG],
                    arow[g_lo * NPG:(g_hi + 1) * NPG]
                    .rearrange("(g s) -> g s", g=ng))
                if l == 2:
                    # v values for these graphs -> interleaved vo columns
                    nc.gpsimd.dma_start(
                        vo[:, 2 * g_lo:2 * (g_hi + 1):2],
                        av_tmp[1, g_lo * NPG:(g_hi + 1) * NPG]
                        .rearrange("(g s) -> s g", g=ng))
            return pT

        def logits(l):
            """Attention ex: exp(lrelu(z)) == max(exp(z), exp(0.2 z)).

            Layer 0's z is the host-folded E0 plane (no matmul); layers 1/2
            build z per chunk from the persistent logits tile + lhs92."""
            ls = lsrc[l % 2]
            ex = expool.tile([NPG, NB], F16, tag="ex")
            for c in range(NCH):
                cs = slice(c * CH, (c + 1) * CH)
                if l == 0:
                    zsrc = E0sb[:, cs]
                else:
                    pl = psb.tile([NPG, CH], F32, tag="pb")
                    nc.tensor.matmul(pl[:], lhs92_sb[:, c * NPG:(c + 1) * NPG],
                                     ls[:, cs], start=True, stop=True)
                    zsrc = pl[:]
                e1 = ltpool.tile([NPG, CH], F16, tag="e1")
                nc.scalar.activation(e1[:], zsrc, AF.Exp)
                e2 = ltpool.tile([NPG, CH], F16, tag="e2")
                nc.scalar.activation(e2[:], zsrc, AF.Exp, scale=NEG_SLOPE)
                nc.vector.tensor_tensor(ex[:, cs], e1[:], e2[:], ALU.max)
            return ex

        hT_in = x_sb
        for l in range(2):
            pT = projection(l, hT_in, 67)
            # node-major [adst | h~ | 1] blocks via per-graph PE transposes.
            # Issued before the logits matmuls so the PE queue has work while
            # the a_src DRAM bounce completes.
            hnode = npool.tile([NPG, GPC * 66], F16, tag="hnode")
            for gs in _graph_banks(GPC, 7):
                pt = pst.tile([NPG, 66 * len(gs)], F16, tag="pt")
                for j, g in enumerate(gs):
                    nc.tensor.transpose(pt[:, j * 66:(j + 1) * 66],
                                        pT[:66, g * NPG:(g + 1) * NPG],
                                        ident_sb[:66, :66])
                nc.vector.tensor_copy(hnode[:, gs[0] * 66:(gs[-1] + 1) * 66], pt[:])
            ex = logits(l)
            # per-graph aggregation: [agg(64) | den] in one matmul
            recip = smpool.tile([NPG, GPC], F32, tag="recip")
            UN = upool.tile([NPG, GPC * HID], F16, tag="UN")
            for gs in _graph_banks(GPC, 7):
                pa = psa.tile([NPG, 65 * len(gs)], F32, tag="pa")
                for j, g in enumerate(gs):
                    exg = ex[:, g * NPG:(g + 1) * NPG]
                    o0 = j * 65
                    nc.tensor.matmul(pa[:, o0:o0 + 65], exg,
                                     hnode[:, g * 66 + 1:g * 66 + 66],
                                     start=True, stop=not use_bias[l])
                    if use_bias[l]:
                        nc.tensor.matmul(pa[:, o0:o0 + 64], exg, bb_sb[l][:],
                                         start=False, stop=True)
                gsl = slice(gs[0], gs[-1] + 1)
                nc.vector.reciprocal(recip[:, gsl], pa[:, 64::65])
                # UN = max(agg, 0) * (1/den), fused (stride-0 bcast)
                pa3 = pa[:].rearrange("p (g c) -> p g c", c=65)[:, :, 0:64]
                un3 = (UN[:, gs[0] * HID:(gs[-1] + 1) * HID]
                       .rearrange("p (g c) -> p g c", c=64))
                rb = _bcast_inner(recip[:, gsl], 64)
                nc.vector.scalar_tensor_tensor(un3, pa3, 0.0, rb,
                                               ALU.max, ALU.mult)
            # transpose pairs back to feature-major [65, NB] (row 64 = ones)
            hT_next = hpool.tile([HID + 1, NB], F16, tag="hT")
            nc.gpsimd.memset(hT_next[HID:HID + 1, :], 1.0)
            pair_banks = _graph_banks(GPC // 2, 6)   # 16 pairs, banks of 6
            for pb in pair_banks:
                ntr = len(pb)
                pt2 = pst2.tile([128, NPG * ntr], F16, tag="pt2")
                for t, pj in enumerate(pb):
                    nc.tensor.transpose(
                        pt2[:, t * NPG:(t + 1) * NPG],
                        UN[:, (2 * pj) * HID:(2 * pj + 2) * HID],
                        ident_sb[:])
                g0 = 2 * pb[0]
                dst = (hT_next[0:HID, :]
                       .rearrange("p (g s) -> p g s", s=NPG))
                src = pt2[:].rearrange("p (t s) -> p t s", s=NPG)
                nc.scalar.copy(dst[:, g0:g0 + 2 * ntr:2, :], src[0:HID])
                nc.vector.tensor_copy(dst[:, g0 + 1:g0 + 2 * ntr:2, :],
                                      src[HID:2 * HID])
            hT_in = hT_next

        # ---- layer 2 (readout folded in) ----
        pT = projection(2, hT_in, 68)
        ex = logits(2)

        pq = psa.tile([NPG, 2 * GPC], F32, tag="pa")
        for g in range(GPC):
            nc.tensor.matmul(pq[:, 2 * g:2 * g + 2],
                             ex[:, g * NPG:(g + 1) * NPG],
                             vo[:, 2 * g:2 * g + 2], start=True, stop=True)
        recip2 = smpool.tile([NPG, GPC], F32, tag="recip")
        nc.vector.reciprocal(recip2[:], pq[:, 1::2])
        qsb = smpool.tile([NPG, GPC], F32, tag="qsb")
        nc.vector.tensor_mul(qsb[:], pq[:, 0::2], recip2[:])
        # exact fp32 pooling: one ones-column matmul sums the partition axis
        zp = pszp.tile([1, GPC], F32, tag="zp")
        nc.tensor.matmul(zp[:], ones84[:], qsb[:], start=True, stop=True)
        zout = smpool.tile([1, GPC], F32, tag="zout")
        nc.scalar.activation(zout[:], zp[:], AF.Relu, bias=float(tail_bias))
        nc.sync.dma_start(out_d.rearrange("(o g) -> o g", o=1), zout[:])

    nc.compile()
    return nc


def _core_inputs(pre, c):
    m = {
        'xT': np.ascontiguousarray(pre['x_aug'][:, c * NB:(c + 1) * NB]),
        'maskc': pre['maskc'], 'lhs92': pre['lhs92'], 'ident': pre['ident'],
    }
    for l in range(DEPTH):
        m[f'E{l}'] = np.ascontiguousarray(
            np.transpose(pre['Es'][l][c * GPC:(c + 1) * GPC], (1, 0, 2))
            .reshape(NPG, NB))
        m[f'CW{l}'] = pre['CW'][l]
    for l in range(2):
        if np.any(pre['bl'][l] != 0):
            m[f'bb{l}'] = np.ascontiguousarray(
                np.tile(pre['bl'][l][None, :], (NPG, 1)).astype(np.float16))
    return m


def kernel(**inputs):
    pre = _host_preprocess(inputs)
    use_bias = tuple(bool(np.any(pre['bl'][l] != 0)) for l in range(2))
    nc = _build_program(pre['tail_bias'], use_bias)
    in_maps = [_core_inputs(pre, c) for c in range(NC_CORES)]
    res = run_bass_kernel_spmd(nc, in_maps, list(range(NC_CORES)))
    out = np.concatenate([np.asarray(res.results[c]['out'])
                          for c in range(NC_CORES)])
    return out.reshape(B, 1).astype(np.float32)


# revision 21
# speedup vs baseline: 1.4041x; 1.0267x over previous
"""Trainium2 Bass kernel for a 3-layer edge-featured GAT over 256 dense 84-node graphs.

Contract: kernel(**inputs) takes the FULL unsharded inputs (as produced by the
problem's setup_inputs) and returns the FULL [256, 1] float32 output.

Strategy (data parallel over graphs, 32 graphs/core on 8 cores):
  Each graph is dense (all ordered pairs + self loops), so message passing
  collapses to dense per-graph [84, 84] attention matrices. Host-side we
  scatter edge_attr into dense per-graph planes (folding the per-layer edge
  MLP down to a scalar per edge, and the PyG mean self-loop attr onto the
  diagonal), fold a_src/a_dst/readout into augmented layer weights, and keep
  a constant-one input feature so every projection carries a ones column
  (which turns the softmax denominator into one extra matmul column).

  Per layer on device: one combined projection produces, per node, the
  projected features h~, a_src/a_dst attention scalars and a constant 1;
  the [src, dst] logit plane is produced by one matmul per 448-col chunk
  whose moving operand is a persistent [92, NB] tile (rows 0:84 = the host
  E plane, DMA'd once per layer; rows 84:91 = block masks, loaded once;
  row 91 = the runtime a_dst row, written by a cheap DVE copy) against a
  stationary that carries identity / runtime a_src rows / ones;
  exp(lrelu(x)) is computed as max(exp(x), exp(0.2 x)); per-graph matmuls
  of ex_g against node-major [h~ | 1] give aggregate + denominator in one
  pass; relu and the 1/den normalization fuse into one strided
  scalar_tensor_tensor per PSUM bank with a stride-0 broadcast AP.

  The final global_add_pool is a single fp32 ones-column matmul over the
  per-node readout (no DRAM transpose bounce), followed by relu+bias.

  All matmul operands are fp16 (PSUM accumulation stays fp32) except the
  tiny exact pooling matmul.
"""

import sys

for _p in ("/opt/trn_rl_repo",):
    if _p not in sys.path:
        sys.path.append(_p)

import numpy as np

from contextlib import ExitStack

from concourse import bacc, bass, mybir, tile
from concourse.bass_types import AP
from concourse.bass_utils import run_bass_kernel_spmd

F32 = mybir.dt.float32
F16 = mybir.dt.float16
AF = mybir.ActivationFunctionType
ALU = mybir.AluOpType

NPG = 84            # nodes per graph
B = 256             # graphs
HID = 64
DEPTH = 3
NEG_SLOPE = 0.2
NC_CORES = 8
GPC = B // NC_CORES     # 32 graphs per core
NB = GPC * NPG          # 2688 nodes per core
CH = 448                # free-dim chunk (one PSUM bank)
NCH = NB // CH          # 6 chunks

# projection column layout: [a_dst | W(64) | ones | a_src | v(layer2)]
C_ADST, C_W0, C_ONE, C_ASRC, C_V = 0, 1, 65, 66, 67


def _chunk_graphs(c):
    """Graphs whose columns intersect chunk c."""
    g_lo = (CH * c) // NPG
    g_hi = (CH * (c + 1) - 1) // NPG
    return g_lo, min(g_hi, GPC - 1)


def _host_preprocess(inputs):
    x = np.ascontiguousarray(np.asarray(inputs['x'], np.float32))
    ei = np.asarray(inputs['edge_index'])
    ea = np.asarray(inputs['edge_attr'], np.float32)
    W0 = np.asarray(inputs['W0'], np.float32)
    Ws = np.asarray(inputs['Ws'], np.float32)
    asl = np.asarray(inputs['att_src_all'], np.float32)
    adl = np.asarray(inputs['att_dst_all'], np.float32)
    Wel = np.asarray(inputs['W_edge_all'], np.float32)
    ael = np.asarray(inputs['att_edge_all'], np.float32)
    bl = np.asarray(inputs['bias_all'], np.float32)
    linW = np.asarray(inputs['lin_W'], np.float32)
    linb = np.asarray(inputs['lin_b'], np.float32)

    src, dst = np.asarray(ei[0]), np.asarray(ei[1])
    g = src // NPG
    assert np.all(dst // NPG == g), "edges cross graph boundaries"
    sl, dl = src % NPG, dst % NPG

    dense = np.zeros((B, NPG, NPG, 2), np.float32)
    dense[g, sl, dl] = ea
    cnt = np.zeros((B, NPG), np.float32)
    np.add.at(cnt, (g, dl), 1.0)
    colsum = dense.sum(axis=1)
    loop_attr = colsum / np.maximum(cnt, 1.0)[..., None]
    di = np.arange(NPG)
    dense[:, di, di, :] = loop_attr

    Es = []
    for l in range(DEPTH):
        w2 = Wel[l] @ ael[l]
        Es.append(np.ascontiguousarray(dense @ w2, dtype=np.float16))
    # layer 0's attention rows depend only on the (known) input x — fold them
    # into the layer-0 E plane so no logits matmul / a_src bounce is needed
    asrc0 = (x[:, 0] * float(W0[0] @ asl[0])).reshape(B, NPG)
    adst0 = (x[:, 0] * float(W0[0] @ adl[0])).reshape(B, NPG)
    Es[0] = np.ascontiguousarray(
        (dense @ (Wel[0] @ ael[0])).astype(np.float32)
        + asrc0[:, :, None] + adst0[:, None, :], np.float16)

    W_all = [W0, Ws[0], Ws[1]]
    CW = []
    for l in range(DEPTH):
        K = W_all[l].shape[0]
        cols = [(W_all[l] @ adl[l])[:, None], W_all[l], np.zeros((K, 1), np.float32),
                (W_all[l] @ asl[l])[:, None]]
        if l == DEPTH - 1:
            cols.append(W_all[l] @ linW)
        A = np.concatenate(cols, axis=1)
        aug = np.zeros((1, A.shape[1]), np.float32)
        aug[0, C_ONE] = 1.0
        CW.append(np.ascontiguousarray(np.vstack([A, aug]), np.float16))

    tail_bias = float(NPG * float(bl[DEPTH - 1] @ linW[:, 0]) + float(linb[0]))

    # per-chunk block-diagonal masks: row k of chunk c covers graph g_lo(c)+k
    maskc = np.zeros((7, NB), np.float16)
    for c in range(NCH):
        g_lo, _ = _chunk_graphs(c)
        for j in range(CH):
            gg = (CH * c + j) // NPG
            maskc[gg - g_lo, CH * c + j] = 1.0
    # merged-logits stationary: row 0 ones (pairs with the runtime a_dst row,
    # which must sit at partition 0 for the DVE row copy), rows 1..84 identity
    # (pairs with E), rows 85..91 runtime a_src rows (pair with maskc)
    lhs92 = np.zeros((92, NCH * NPG), np.float16)
    lhs92[0, :] = 1.0
    for c in range(NCH):
        lhs92[1:NPG + 1, c * NPG:(c + 1) * NPG] = np.eye(NPG, dtype=np.float16)
    ident = np.eye(NPG, dtype=np.float16)
    x_aug = np.ones((2, B * NPG), np.float16)
    x_aug[0] = x[:, 0].astype(np.float16)

    return dict(x_aug=x_aug, Es=Es, CW=CW, bl=bl, tail_bias=tail_bias,
                maskc=maskc, lhs92=lhs92, ident=ident)


def _graph_banks(n_graphs, per_bank):
    out = []
    g0 = 0
    while g0 < n_graphs:
        out.append(list(range(g0, min(g0 + per_bank, n_graphs))))
        g0 += per_bank
    return out


def _bcast_inner(ap, n):
    """View `ap` with an extra innermost stride-0 axis of length n."""
    return AP(ap.tensor, ap.offset, list(ap.ap) + [[0, n]])


def _build_program(tail_bias, use_bias):
    """use_bias: (bool, bool) for layers 0 and 1 (per-node bias via ex@bb matmul)."""
    nc = bacc.Bacc("TRN2", target_bir_lowering=False, debug=False)

    xT_d = nc.dram_tensor("xT", [2, NB], F16, kind="ExternalInput").ap()
    E_d = [nc.dram_tensor(f"E{l}", [NPG, NB], F16, kind="ExternalInput").ap()
           for l in range(DEPTH)]
    ncw = [67, 67, 68]
    CW_d = [nc.dram_tensor(f"CW{l}", [(2 if l == 0 else HID + 1), ncw[l]],
                           F16, kind="ExternalInput").ap() for l in range(DEPTH)]
    maskc_d = nc.dram_tensor("maskc", [7, NB], F16, kind="ExternalInput").ap()
    lhs92_d = nc.dram_tensor("lhs92", [92, NCH * NPG], F16, kind="ExternalInput").ap()
    ident_d = nc.dram_tensor("ident", [NPG, NPG], F16, kind="ExternalInput").ap()
    bb_d = [nc.dram_tensor(f"bb{l}", [NPG, HID], F16, kind="ExternalInput").ap()
            if use_bias[l] else None for l in range(2)]
    # row bounce scratch (sbuf row -> dram -> repartitioned sbuf)
    asrc_tmp = [nc.dram_tensor(f"asrc_tmp{l}", [NB], F16).ap() for l in range(2)]
    av_tmp = nc.dram_tensor("av_tmp", [2, NB], F16).ap()   # layer2: [asrc; v]
    out_d = nc.dram_tensor("out", [GPC], F32, kind="ExternalOutput").ap()

    with tile.TileContext(nc) as tc, ExitStack() as ctx:
        cpool = ctx.enter_context(tc.tile_pool(name="const", bufs=1))
        hpool = ctx.enter_context(tc.tile_pool(name="h", bufs=2))
        ppool = ctx.enter_context(tc.tile_pool(name="proj", bufs=2))
        npool = ctx.enter_context(tc.tile_pool(name="hnode", bufs=2))
        expool = ctx.enter_context(tc.tile_pool(name="ex", bufs=2))
        ltpool = ctx.enter_context(tc.tile_pool(name="lt", bufs=3))
        smpool = ctx.enter_context(tc.tile_pool(name="small", bufs=3))
        upool = ctx.enter_context(tc.tile_pool(name="u", bufs=2))

        psb = ctx.enter_context(tc.tile_pool(name="psb", bufs=2, space="PSUM"))
        psa = ctx.enter_context(tc.tile_pool(name="psa", bufs=2, space="PSUM"))
        pst = ctx.enter_context(tc.tile_pool(name="pst", bufs=1, space="PSUM"))
        pst2 = ctx.enter_context(tc.tile_pool(name="pst2", bufs=2, space="PSUM"))
        pszp = ctx.enter_context(tc.tile_pool(name="pszp", bufs=1, space="PSUM"))

        # constants — small critical inputs (x, CW, ident, lhs92) first so the
        # first projection/transpose work isn't queued behind the E planes
        x_sb = hpool.tile([2, NB], F16, tag="x")
        nc.sync.dma_start(x_sb[:], xT_d[:])
        cw_sb = []
        for l in range(DEPTH):
            t = cpool.tile(list(CW_d[l].shape), F16, tag=f"cw{l}")
            nc.sync.dma_start(t[:], CW_d[l][:])
            cw_sb.append(t)
        ident_sb = cpool.tile([NPG, NPG], F16, tag="ident")
        nc.sync.dma_start(ident_sb[:], ident_d[:])
        lhs92_sb = cpool.tile([92, NCH * NPG], F16, tag="lhs92")
        nc.sync.dma_start(lhs92_sb[:], lhs92_d[:])
        bb_sb = []
        for l in range(2):
            if use_bias[l]:
                t = cpool.tile([NPG, HID], F16, tag=f"bb{l}")
                nc.sync.dma_start(t[:], bb_d[l][:])
                bb_sb.append(t)
            else:
                bb_sb.append(None)

        # layer 0: fully folded E plane (E + a_src + a_dst), exp'd directly.
        # Loaded in thirds so the first exp chunks start as early as possible.
        E0sb = cpool.tile([NPG, NB], F16, tag="E0sb")
        for i in range(3):
            s = slice(i * (NB // 3), (i + 1) * (NB // 3))
            nc.sync.dma_start(E0sb[:, s], E_d[0][:, s])
        # persistent logits moving-operand tiles for layers 1/2: row 0 =
        # runtime a_dst (per chunk), rows 1:85 = E, rows 85:92 = maskc (once)
        lsrc = [cpool.tile([92, NB], F16, tag=f"lsrc{i}", name=f"lsrc{i}")
                for i in range(2)]
        for i in range(2):
            nc.gpsimd.dma_start(lsrc[i][NPG + 1:NPG + 8, :], maskc_d[:])
            nc.sync.dma_start(lsrc[i][1:NPG + 1, :], E_d[2 - i][:])

        # layer-2 interleaved [v | 1] aggregation operand; ones set up front
        vo = smpool.tile([NPG, 2 * GPC], F16, tag="vo")
        nc.gpsimd.memset(vo[:], 1.0)
        # fp32 ones column for the exact pooling matmul
        ones84 = smpool.tile([NPG, 1], F32, tag="ones84")
        nc.vector.memset(ones84[:], 1.0)

        def projection(l, hT_in, nrows):
            """Projection + a_src/a_dst row distribution. Returns pT."""
            ls = lsrc[l % 2]
            pT = ppool.tile([nrows, NB], F16, tag="pT")
            for c in range(NCH):
                cs = slice(c * CH, (c + 1) * CH)
                pw = psb.tile([nrows, CH], F32, tag="pb")
                nc.tensor.matmul(pw[:], cw_sb[l][:], hT_in[:, cs],
                                 start=True, stop=True)
                if c % 2 == 0:
                    nc.scalar.copy(pT[:, cs], pw[:])
                else:
                    nc.vector.tensor_copy(pT[:, cs], pw[:])
                if l == 0:
                    continue    # layer 0 attention rows are host-folded
                # a_dst row into the persistent logits tile (cheap DVE row copy)
                nc.vector.tensor_copy(ls[0:1, cs], pT[C_ADST:C_ADST + 1, cs])
                # bounce this chunk's a_src row immediately (plus v for layer 2)
                if l < 2:
                    nc.sync.dma_start(
                        asrc_tmp[l][cs].rearrange("(o n) -> o n", o=1),
                        pT[C_ASRC:C_ASRC + 1, cs])
                else:
                    nc.sync.dma_start(av_tmp[:, cs], pT[C_ASRC:C_V + 1, cs])
            for c in range(NCH if l > 0 else 0):
                g_lo, g_hi = _chunk_graphs(c)
                ng = g_hi - g_lo + 1
                arow = asrc_tmp[l] if l < 2 else av_tmp[0]
                nc.sync.dma_start(
                    lhs92_sb[85:85 + ng, c * NPG:(c + 1) * NPG],
                    arow[g_lo * NPG:(g_hi + 1) * NPG]
                    .rearrange("(g s) -> g s", g=ng))
                if l == 2:
                    # v values for these graphs -> interleaved vo columns
                    nc.sync.dma_start(
                        vo[:, 2 * g_lo:2 * (g_hi + 1):2],
                        av_tmp[1, g_lo * NPG:(g_hi + 1) * NPG]
                        .rearrange("(g s) -> s g", g=ng))
            return pT

        def logits(l):
            """Attention ex: exp(lrelu(z)) == max(exp(z), exp(0.2 z)).

            Layer 0's z is the host-folded E0 plane (no matmul); layers 1/2
            build z per chunk from the persistent logits tile + lhs92."""
            ls = lsrc[l % 2]
            ex = expool.tile([NPG, NB], F16, tag="ex")
            for c in range(NCH):
                cs = slice(c * CH, (c + 1) * CH)
                if l == 0:
                    zsrc = E0sb[:, cs]
                else:
                    pl = psb.tile([NPG, CH], F32, tag="pb")
                    nc.tensor.matmul(pl[:], lhs92_sb[:, c * NPG:(c + 1) * NPG],
                                     ls[:, cs], start=True, stop=True)
                    zsrc = pl[:]
                e1 = ltpool.tile([NPG, CH], F16, tag="e1")
                nc.scalar.activation(e1[:], zsrc, AF.Exp)
                e2 = ltpool.tile([NPG, CH], F16, tag="e2")
                nc.scalar.activation(e2[:], zsrc, AF.Exp, scale=NEG_SLOPE)
                nc.vector.tensor_tensor(ex[:, cs], e1[:], e2[:], ALU.max)
            return ex

        hT_in = x_sb
        for l in range(2):
            pT = projection(l, hT_in, 67)
            # node-major [adst | h~ | 1] blocks via per-graph PE transposes.
            # Issued before the logits matmuls so the PE queue has work while
            # the a_src DRAM bounce completes.
            hnode = npool.tile([NPG, GPC * 66], F16, tag="hnode")
            for gs in _graph_banks(GPC, 7):
                pt = pst.tile([NPG, 66 * len(gs)], F16, tag="pt")
                for j, g in enumerate(gs):
                    nc.tensor.transpose(pt[:, j * 66:(j + 1) * 66],
                                        pT[:66, g * NPG:(g + 1) * NPG],
                                        ident_sb[:66, :66])
                nc.vector.tensor_copy(hnode[:, gs[0] * 66:(gs[-1] + 1) * 66], pt[:])
            ex = logits(l)
            # per-graph aggregation: [agg(64) | den] in one matmul
            recip = smpool.tile([NPG, GPC], F32, tag="recip")
            UN = upool.tile([NPG, GPC * HID], F16, tag="UN")
            for gs in _graph_banks(GPC, 7):
                pa = psa.tile([NPG, 65 * len(gs)], F32, tag="pa")
                for j, g in enumerate(gs):
                    exg = ex[:, g * NPG:(g + 1) * NPG]
                    o0 = j * 65
                    nc.tensor.matmul(pa[:, o0:o0 + 65], exg,
                                     hnode[:, g * 66 + 1:g * 66 + 66],
                                     start=True, stop=not use_bias[l])
                    if use_bias[l]:
                        nc.tensor.matmul(pa[:, o0:o0 + 64], exg, bb_sb[l][:],
                                         start=False, stop=True)
                gsl = slice(gs[0], gs[-1] + 1)
                nc.vector.reciprocal(recip[:, gsl], pa[:, 64::65])
                # UN = max(agg, 0) * (1/den), fused (stride-0 bcast)
                pa3 = pa[:].rearrange("p (g c) -> p g c", c=65)[:, :, 0:64]
                un3 = (UN[:, gs[0] * HID:(gs[-1] + 1) * HID]
                       .rearrange("p (g c) -> p g c", c=64))
                rb = _bcast_inner(recip[:, gsl], 64)
                nc.vector.scalar_tensor_tensor(un3, pa3, 0.0, rb,
                                               ALU.max, ALU.mult)
            # transpose pairs back to feature-major [65, NB] (row 64 = ones)
            hT_next = hpool.tile([HID + 1, NB], F16, tag="hT")
            nc.gpsimd.memset(hT_next[HID:HID + 1, :], 1.0)
            pair_banks = _graph_banks(GPC // 2, 6)   # 16 pairs, banks of 6
            for pb in pair_banks:
                ntr = len(pb)
                pt2 = pst2.tile([128, NPG * ntr], F16, tag="pt2")
                for t, pj in enumerate(pb):
                    nc.tensor.transpose(
                        pt2[:, t * NPG:(t + 1) * NPG],
                        UN[:, (2 * pj) * HID:(2 * pj + 2) * HID],
                        ident_sb[:])
                g0 = 2 * pb[0]
                dst = (hT_next[0:HID, :]
                       .rearrange("p (g s) -> p g s", s=NPG))
                src = pt2[:].rearrange("p (t s) -> p t s", s=NPG)
                nc.scalar.copy(dst[:, g0:g0 + 2 * ntr:2, :], src[0:HID])
                nc.vector.tensor_copy(dst[:, g0 + 1:g0 + 2 * ntr:2, :],
                                      src[HID:2 * HID])
            hT_in = hT_next

        # ---- layer 2 (readout folded in) ----
        pT = projection(2, hT_in, 68)
        ex = logits(2)

        pq = psa.tile([NPG, 2 * GPC], F32, tag="pa")
        for g in range(GPC):
            nc.tensor.matmul(pq[:, 2 * g:2 * g + 2],
                             ex[:, g * NPG:(g + 1) * NPG],
                             vo[:, 2 * g:2 * g + 2], start=True, stop=True)
        recip2 = smpool.tile([NPG, GPC], F32, tag="recip")
        nc.vector.reciprocal(recip2[:], pq[:, 1::2])
        qsb = smpool.tile([NPG, GPC], F32, tag="qsb")
        nc.vector.tensor_mul(qsb[:], pq[:, 0::2], recip2[:])
        # exact fp32 pooling: one ones-column matmul sums the partition axis
        zp = pszp.tile([1, GPC], F32, tag="zp")
        nc.tensor.matmul(zp[:], ones84[:], qsb[:], start=True, stop=True)
        zout = smpool.tile([1, GPC], F32, tag="zout")
        nc.scalar.activation(zout[:], zp[:], AF.Relu, bias=float(tail_bias))
        nc.sync.dma_start(out_d.rearrange("(o g) -> o g", o=1), zout[:])

    nc.compile()
    return nc


def _core_inputs(pre, c):
    m = {
        'xT': np.ascontiguousarray(pre['x_aug'][:, c * NB:(c + 1) * NB]),
        'maskc': pre['maskc'], 'lhs92': pre['lhs92'], 'ident': pre['ident'],
    }
    for l in range(DEPTH):
        m[f'E{l}'] = np.ascontiguousarray(
            np.transpose(pre['Es'][l][c * GPC:(c + 1) * GPC], (1, 0, 2))
            .reshape(NPG, NB))
        m[f'CW{l}'] = pre['CW'][l]
    for l in range(2):
        if np.any(pre['bl'][l] != 0):
            m[f'bb{l}'] = np.ascontiguousarray(
                np.tile(pre['bl'][l][None, :], (NPG, 1)).astype(np.float16))
    return m


def kernel(**inputs):
    pre = _host_preprocess(inputs)
    use_bias = tuple(bool(np.any(pre['bl'][l] != 0)) for l in range(2))
    nc = _build_program(pre['tail_bias'], use_bias)
    in_maps = [_core_inputs(pre, c) for c in range(NC_CORES)]
    res = run_bass_kernel_spmd(nc, in_maps, list(range(NC_CORES)))
    out = np.concatenate([np.asarray(res.results[c]['out'])
                          for c in range(NC_CORES)])
    return out.reshape(B, 1).astype(np.float32)


# revision 25
# speedup vs baseline: 1.4108x; 1.0047x over previous
"""Trainium2 Bass kernel for a 3-layer edge-featured GAT over 256 dense 84-node graphs.

Contract: kernel(**inputs) takes the FULL unsharded inputs (as produced by the
problem's setup_inputs) and returns the FULL [256, 1] float32 output.

Strategy (data parallel over graphs, 32 graphs/core on 8 cores):
  Each graph is dense (all ordered pairs + self loops), so message passing
  collapses to dense per-graph [84, 84] attention matrices. Host-side we
  scatter edge_attr into dense per-graph planes (folding the per-layer edge
  MLP down to a scalar per edge, and the PyG mean self-loop attr onto the
  diagonal), fold a_src/a_dst/readout into augmented layer weights, and keep
  a constant-one input feature so every projection carries a ones column
  (which turns the softmax denominator into one extra matmul column).

  Per layer on device: one combined projection produces, per node, the
  projected features h~, a_src/a_dst attention scalars and a constant 1;
  the [src, dst] logit plane is produced by one matmul per 448-col chunk
  whose moving operand is a persistent [92, NB] tile (rows 0:84 = the host
  E plane, DMA'd once per layer; rows 84:91 = block masks, loaded once;
  row 91 = the runtime a_dst row, written by a cheap DVE copy) against a
  stationary that carries identity / runtime a_src rows / ones;
  exp(lrelu(x)) is computed as max(exp(x), exp(0.2 x)); per-graph matmuls
  of ex_g against node-major [h~ | 1] give aggregate + denominator in one
  pass; relu and the 1/den normalization fuse into one strided
  scalar_tensor_tensor per PSUM bank with a stride-0 broadcast AP.

  The final global_add_pool is a single fp32 ones-column matmul over the
  per-node readout (no DRAM transpose bounce), followed by relu+bias.

  All matmul operands are fp16 (PSUM accumulation stays fp32) except the
  tiny exact pooling matmul.
"""

import sys

for _p in ("/opt/trn_rl_repo",):
    if _p not in sys.path:
        sys.path.append(_p)

import numpy as np

from contextlib import ExitStack

from concourse import bacc, bass, mybir, tile
from concourse.bass_types import AP
from concourse.bass_utils import run_bass_kernel_spmd

F32 = mybir.dt.float32
F16 = mybir.dt.float16
AF = mybir.ActivationFunctionType
ALU = mybir.AluOpType

NPG = 84            # nodes per graph
B = 256             # graphs
HID = 64
DEPTH = 3
NEG_SLOPE = 0.2
NC_CORES = 8
GPC = B // NC_CORES     # 32 graphs per core
NB = GPC * NPG          # 2688 nodes per core
CH = 448                # free-dim chunk (one PSUM bank)
NCH = NB // CH          # 6 chunks

# projection column layout: [a_dst | W(64) | ones | a_src | v(layer2)]
C_ADST, C_W0, C_ONE, C_ASRC, C_V = 0, 1, 65, 66, 67


def _chunk_graphs(c):
    """Graphs whose columns intersect chunk c."""
    g_lo = (CH * c) // NPG
    g_hi = (CH * (c + 1) - 1) // NPG
    return g_lo, min(g_hi, GPC - 1)


def _host_preprocess(inputs):
    x = np.ascontiguousarray(np.asarray(inputs['x'], np.float32))
    ei = np.asarray(inputs['edge_index'])
    ea = np.asarray(inputs['edge_attr'], np.float32)
    W0 = np.asarray(inputs['W0'], np.float32)
    Ws = np.asarray(inputs['Ws'], np.float32)
    asl = np.asarray(inputs['att_src_all'], np.float32)
    adl = np.asarray(inputs['att_dst_all'], np.float32)
    Wel = np.asarray(inputs['W_edge_all'], np.float32)
    ael = np.asarray(inputs['att_edge_all'], np.float32)
    bl = np.asarray(inputs['bias_all'], np.float32)
    linW = np.asarray(inputs['lin_W'], np.float32)
    linb = np.asarray(inputs['lin_b'], np.float32)

    src, dst = np.asarray(ei[0]), np.asarray(ei[1])
    g = src // NPG
    assert np.all(dst // NPG == g), "edges cross graph boundaries"
    sl, dl = src % NPG, dst % NPG

    dense = np.zeros((B, NPG, NPG, 2), np.float32)
    dense[g, sl, dl] = ea
    cnt = np.zeros((B, NPG), np.float32)
    np.add.at(cnt, (g, dl), 1.0)
    colsum = dense.sum(axis=1)
    loop_attr = colsum / np.maximum(cnt, 1.0)[..., None]
    di = np.arange(NPG)
    dense[:, di, di, :] = loop_attr

    Es = []
    for l in range(DEPTH):
        w2 = Wel[l] @ ael[l]
        Es.append(np.ascontiguousarray(dense @ w2, dtype=np.float16))
    # layer 0's attention rows depend only on the (known) input x — fold them
    # into the layer-0 E plane so no logits matmul / a_src bounce is needed
    asrc0 = (x[:, 0] * float(W0[0] @ asl[0])).reshape(B, NPG)
    adst0 = (x[:, 0] * float(W0[0] @ adl[0])).reshape(B, NPG)
    Es[0] = np.ascontiguousarray(
        (dense @ (Wel[0] @ ael[0])).astype(np.float32)
        + asrc0[:, :, None] + adst0[:, None, :], np.float16)

    W_all = [W0, Ws[0], Ws[1]]
    CW = []
    for l in range(DEPTH):
        K = W_all[l].shape[0]
        cols = [(W_all[l] @ adl[l])[:, None], W_all[l], np.zeros((K, 1), np.float32),
                (W_all[l] @ asl[l])[:, None]]
        if l == DEPTH - 1:
            cols.append(W_all[l] @ linW)
        A = np.concatenate(cols, axis=1)
        aug = np.zeros((1, A.shape[1]), np.float32)
        aug[0, C_ONE] = 1.0
        CW.append(np.ascontiguousarray(np.vstack([A, aug]), np.float16))

    tail_bias = float(NPG * float(bl[DEPTH - 1] @ linW[:, 0]) + float(linb[0]))

    # per-chunk block-diagonal masks: row k of chunk c covers graph g_lo(c)+k
    maskc = np.zeros((7, NB), np.float16)
    for c in range(NCH):
        g_lo, _ = _chunk_graphs(c)
        for j in range(CH):
            gg = (CH * c + j) // NPG
            maskc[gg - g_lo, CH * c + j] = 1.0
    # merged-logits stationary: row 0 ones (pairs with the runtime a_dst row,
    # which must sit at partition 0 for the DVE row copy), rows 1..84 identity
    # (pairs with E), rows 85..91 runtime a_src rows (pair with maskc)
    lhs92 = np.zeros((92, NCH * NPG), np.float16)
    lhs92[0, :] = 1.0
    for c in range(NCH):
        lhs92[1:NPG + 1, c * NPG:(c + 1) * NPG] = np.eye(NPG, dtype=np.float16)
    ident = np.eye(NPG, dtype=np.float16)
    x_aug = np.ones((2, B * NPG), np.float16)
    x_aug[0] = x[:, 0].astype(np.float16)

    return dict(x_aug=x_aug, Es=Es, CW=CW, bl=bl, tail_bias=tail_bias,
                maskc=maskc, lhs92=lhs92, ident=ident)


def _graph_banks(n_graphs, per_bank):
    out = []
    g0 = 0
    while g0 < n_graphs:
        out.append(list(range(g0, min(g0 + per_bank, n_graphs))))
        g0 += per_bank
    return out


def _bcast_inner(ap, n):
    """View `ap` with an extra innermost stride-0 axis of length n."""
    return AP(ap.tensor, ap.offset, list(ap.ap) + [[0, n]])


def _build_program(tail_bias, use_bias):
    """use_bias: (bool, bool) for layers 0 and 1 (per-node bias via ex@bb matmul)."""
    nc = bacc.Bacc("TRN2", target_bir_lowering=False, debug=False)

    xT_d = nc.dram_tensor("xT", [2, NB], F16, kind="ExternalInput").ap()
    E_d = [nc.dram_tensor(f"E{l}", [NPG, NB], F16, kind="ExternalInput").ap()
           for l in range(DEPTH)]
    ncw = [67, 67, 68]
    CW_d = [nc.dram_tensor(f"CW{l}", [(2 if l == 0 else HID + 1), ncw[l]],
                           F16, kind="ExternalInput").ap() for l in range(DEPTH)]
    maskc_d = nc.dram_tensor("maskc", [7, NB], F16, kind="ExternalInput").ap()
    lhs92_d = nc.dram_tensor("lhs92", [92, NCH * NPG], F16, kind="ExternalInput").ap()
    ident_d = nc.dram_tensor("ident", [NPG, NPG], F16, kind="ExternalInput").ap()
    bb_d = [nc.dram_tensor(f"bb{l}", [NPG, HID], F16, kind="ExternalInput").ap()
            if use_bias[l] else None for l in range(2)]
    # row bounce scratch (sbuf row -> dram -> repartitioned sbuf)
    asrc_tmp = [nc.dram_tensor(f"asrc_tmp{l}", [NB], F16).ap() for l in range(2)]
    av_tmp = nc.dram_tensor("av_tmp", [2, NB], F16).ap()   # layer2: [asrc; v]
    out_d = nc.dram_tensor("out", [GPC], F32, kind="ExternalOutput").ap()

    with tile.TileContext(nc) as tc, ExitStack() as ctx:
        cpool = ctx.enter_context(tc.tile_pool(name="const", bufs=1))
        hpool = ctx.enter_context(tc.tile_pool(name="h", bufs=2))
        ppool = ctx.enter_context(tc.tile_pool(name="proj", bufs=2))
        npool = ctx.enter_context(tc.tile_pool(name="hnode", bufs=2))
        expool = ctx.enter_context(tc.tile_pool(name="ex", bufs=2))
        ltpool = ctx.enter_context(tc.tile_pool(name="lt", bufs=3))
        smpool = ctx.enter_context(tc.tile_pool(name="small", bufs=3))
        upool = ctx.enter_context(tc.tile_pool(name="u", bufs=2))

        # psb holds 2-bank-wide [84, 1024] tiles: two 448-col matmuls land at
        # columns 0 and 512 (bank-aligned), then evac/exp process 896 at once
        psb = ctx.enter_context(tc.tile_pool(name="psb", bufs=2, space="PSUM"))
        psa = ctx.enter_context(tc.tile_pool(name="psa", bufs=2, space="PSUM"))
        pst = ctx.enter_context(tc.tile_pool(name="pst", bufs=1, space="PSUM"))
        pst2 = ctx.enter_context(tc.tile_pool(name="pst2", bufs=1, space="PSUM"))

        # constants — small critical inputs (x, CW, ident, lhs92) first so the
        # first projection/transpose work isn't queued behind the E planes
        x_sb = hpool.tile([2, NB], F16, tag="x")
        nc.sync.dma_start(x_sb[:], xT_d[:])
        cw_sb = []
        for l in range(DEPTH):
            t = cpool.tile(list(CW_d[l].shape), F16, tag=f"cw{l}")
            nc.sync.dma_start(t[:], CW_d[l][:])
            cw_sb.append(t)
        ident_sb = cpool.tile([NPG, NPG], F16, tag="ident")
        nc.sync.dma_start(ident_sb[:], ident_d[:])
        lhs92_sb = cpool.tile([92, NCH * NPG], F16, tag="lhs92")
        nc.sync.dma_start(lhs92_sb[:], lhs92_d[:])
        bb_sb = []
        for l in range(2):
            if use_bias[l]:
                t = cpool.tile([NPG, HID], F16, tag=f"bb{l}")
                nc.sync.dma_start(t[:], bb_d[l][:])
                bb_sb.append(t)
            else:
                bb_sb.append(None)

        # layer 0: fully folded E plane (E + a_src + a_dst), exp'd directly.
        # Loaded in thirds so the first exp chunks start as early as possible.
        E0sb = cpool.tile([NPG, NB], F16, tag="E0sb")
        for i in range(3):
            s = slice(i * (NB // 3), (i + 1) * (NB // 3))
            nc.sync.dma_start(E0sb[:, s], E_d[0][:, s])
        # persistent logits moving-operand tiles for layers 1/2: row 0 =
        # runtime a_dst (per chunk), rows 1:85 = E, rows 85:92 = maskc (once)
        lsrc = [cpool.tile([92, NB], F16, tag=f"lsrc{i}", name=f"lsrc{i}")
                for i in range(2)]
        for i in range(2):
            nc.gpsimd.dma_start(lsrc[i][NPG + 1:NPG + 8, :], maskc_d[:])
            nc.sync.dma_start(lsrc[i][1:NPG + 1, :], E_d[2 - i][:])

        # layer-2 interleaved [v | 1] aggregation operand; ones set up front
        vo = smpool.tile([NPG, 2 * GPC], F16, tag="vo")
        nc.gpsimd.memset(vo[:], 1.0)
        # fp32 ones column for the exact pooling matmul
        ones84 = smpool.tile([NPG, 1], F32, tag="ones84")
        nc.vector.memset(ones84[:], 1.0)

        CHW = 2 * CH      # 896: two chunks per PSUM-wide super-chunk

        def projection(l, hT_in, nrows):
            """Projection + a_src/a_dst row distribution. Returns pT."""
            ls = lsrc[l % 2]
            pT = ppool.tile([nrows, NB], F16, tag="pT")
            for sc in range(NCH // 2):
                ss = slice(sc * CHW, (sc + 1) * CHW)
                pw = psb.tile([NPG, 1024], F32, tag="pb")
                for h in range(2):
                    cs = slice((2 * sc + h) * CH, (2 * sc + h + 1) * CH)
                    nc.tensor.matmul(pw[:nrows, 512 * h:512 * h + CH],
                                     cw_sb[l][:], hT_in[:, cs],
                                     start=True, stop=True)
                src = (pw[:nrows].rearrange("p (h c) -> p h c", c=512)
                       [:, :, 0:CH])
                dst = pT[:, ss].rearrange("p (h c) -> p h c", c=CH)
                if sc % 2 == 1:
                    nc.scalar.copy(dst, src)
                else:
                    nc.vector.tensor_copy(dst, src)
                if l == 0:
                    continue    # layer 0 attention rows are host-folded
                # a_dst row into the persistent logits tile (cheap DVE row copy)
                nc.vector.tensor_copy(ls[0:1, ss], pT[C_ADST:C_ADST + 1, ss])
                # bounce this super-chunk's a_src row (plus v for layer 2)
                if l < 2:
                    nc.sync.dma_start(
                        asrc_tmp[l][ss].rearrange("(o n) -> o n", o=1),
                        pT[C_ASRC:C_ASRC + 1, ss])
                else:
                    nc.sync.dma_start(av_tmp[:, ss], pT[C_ASRC:C_V + 1, ss])
            for c in range(NCH if l > 0 else 0):
                g_lo, g_hi = _chunk_graphs(c)
                ng = g_hi - g_lo + 1
                arow = asrc_tmp[l] if l < 2 else av_tmp[0]
                nc.sync.dma_start(
                    lhs92_sb[85:85 + ng, c * NPG:(c + 1) * NPG],
                    arow[g_lo * NPG:(g_hi + 1) * NPG]
                    .rearrange("(g s) -> g s", g=ng))
                if l == 2:
                    # v values for these graphs -> interleaved vo columns
                    nc.sync.dma_start(
                        vo[:, 2 * g_lo:2 * (g_hi + 1):2],
                        av_tmp[1, g_lo * NPG:(g_hi + 1) * NPG]
                        .rearrange("(g s) -> s g", g=ng))
            return pT

        def logits(l):
            """Attention ex: exp(lrelu(z)) == max(exp(z), exp(0.2 z)).

            Layer 0's z is the host-folded E0 plane (no matmul); layers 1/2
            build z per chunk from the persistent logits tile + lhs92."""
            ls = lsrc[l % 2]
            ex = expool.tile([NPG, NB], F16, tag="ex")
            for sc in range(NCH // 2):
                ss = slice(sc * CHW, (sc + 1) * CHW)
                if l == 0:
                    zsrc = E0sb[:, ss]
                else:
                    pl = psb.tile([NPG, 1024], F32, tag="pb")
                    for h in range(2):
                        c = 2 * sc + h
                        nc.tensor.matmul(pl[:, 512 * h:512 * h + CH],
                                         lhs92_sb[:, c * NPG:(c + 1) * NPG],
                                         ls[:, c * CH:(c + 1) * CH],
                                         start=True, stop=True)
                    zsrc = (pl[:].rearrange("p (h c) -> p h c", c=512)
                            [:, :, 0:CH])
                e1 = ltpool.tile([NPG, CHW], F16, tag="e1")
                e1d = e1[:].rearrange("p (h c) -> p h c", c=CH) if l else e1[:]
                nc.scalar.activation(e1d, zsrc, AF.Exp)
                e2 = ltpool.tile([NPG, CHW], F16, tag="e2")
                e2d = e2[:].rearrange("p (h c) -> p h c", c=CH) if l else e2[:]
                nc.scalar.activation(e2d, zsrc, AF.Exp, scale=NEG_SLOPE)
                nc.vector.tensor_tensor(ex[:, ss], e1[:], e2[:], ALU.max)
            return ex

        hT_in = x_sb
        for l in range(2):
            pT = projection(l, hT_in, 67)
            # node-major [adst | h~ | 1] blocks via per-graph PE transposes.
            # Issued before the logits matmuls so the PE queue has work while
            # the a_src DRAM bounce completes.
            hnode = npool.tile([NPG, GPC * 66], F16, tag="hnode")
            for gs in _graph_banks(GPC, 7):
                pt = pst.tile([NPG, 66 * len(gs)], F16, tag="pt")
                for j, g in enumerate(gs):
                    nc.tensor.transpose(pt[:, j * 66:(j + 1) * 66],
                                        pT[:66, g * NPG:(g + 1) * NPG],
                                        ident_sb[:66, :66])
                nc.vector.tensor_copy(hnode[:, gs[0] * 66:(gs[-1] + 1) * 66], pt[:])
            ex = logits(l)
            # per-graph aggregation: [agg(64) | den] in one matmul
            recip = smpool.tile([NPG, GPC], F32, tag="recip")
            UN = upool.tile([NPG, GPC * HID], F16, tag="UN")
            for gs in _graph_banks(GPC, 7):
                pa = psa.tile([NPG, 65 * len(gs)], F32, tag="pa")
                for j, g in enumerate(gs):
                    exg = ex[:, g * NPG:(g + 1) * NPG]
                    o0 = j * 65
                    nc.tensor.matmul(pa[:, o0:o0 + 65], exg,
                                     hnode[:, g * 66 + 1:g * 66 + 66],
                                     start=True, stop=not use_bias[l])
                    if use_bias[l]:
                        nc.tensor.matmul(pa[:, o0:o0 + 64], exg, bb_sb[l][:],
                                         start=False, stop=True)
                gsl = slice(gs[0], gs[-1] + 1)
                nc.vector.reciprocal(recip[:, gsl], pa[:, 64::65])
                # UN = max(agg, 0) * (1/den), fused (stride-0 bcast)
                pa3 = pa[:].rearrange("p (g c) -> p g c", c=65)[:, :, 0:64]
                un3 = (UN[:, gs[0] * HID:(gs[-1] + 1) * HID]
                       .rearrange("p (g c) -> p g c", c=64))
                rb = _bcast_inner(recip[:, gsl], 64)
                nc.vector.scalar_tensor_tensor(un3, pa3, 0.0, rb,
                                               ALU.max, ALU.mult)
            # transpose pairs back to feature-major [65, NB] (row 64 = ones)
            hT_next = hpool.tile([HID + 1, NB], F16, tag="hT")
            nc.gpsimd.memset(hT_next[HID:HID + 1, :], 1.0)
            pair_banks = _graph_banks(GPC // 2, 6)   # 16 pairs, banks of 6
            for pb in pair_banks:
                ntr = len(pb)
                pt2 = pst2.tile([128, NPG * ntr], F16, tag="pt2")
                for t, pj in enumerate(pb):
                    nc.tensor.transpose(
                        pt2[:, t * NPG:(t + 1) * NPG],
                        UN[:, (2 * pj) * HID:(2 * pj + 2) * HID],
                        ident_sb[:])
                g0 = 2 * pb[0]
                dst = (hT_next[0:HID, :]
                       .rearrange("p (g s) -> p g s", s=NPG))
                src = pt2[:].rearrange("p (t s) -> p t s", s=NPG)
                nc.scalar.copy(dst[:, g0:g0 + 2 * ntr:2, :], src[0:HID])
                nc.vector.tensor_copy(dst[:, g0 + 1:g0 + 2 * ntr:2, :],
                                      src[HID:2 * HID])
            hT_in = hT_next

        # ---- layer 2 (readout folded in) ----
        pT = projection(2, hT_in, 68)
        ex = logits(2)

        pq = psa.tile([NPG, 2 * GPC], F32, tag="pa")
        for g in range(GPC):
            nc.tensor.matmul(pq[:, 2 * g:2 * g + 2],
                             ex[:, g * NPG:(g + 1) * NPG],
                             vo[:, 2 * g:2 * g + 2], start=True, stop=True)
        recip2 = smpool.tile([NPG, GPC], F32, tag="recip")
        nc.vector.reciprocal(recip2[:], pq[:, 1::2])
        qsb = smpool.tile([NPG, GPC], F32, tag="qsb")
        nc.vector.tensor_mul(qsb[:], pq[:, 0::2], recip2[:])
        # exact fp32 pooling: one ones-column matmul sums the partition axis
        # (reuses a psa slot; only rows [0:1, 0:GPC] are written/read)
        zp_t = psa.tile([NPG, 65 * 7], F32, tag="pa")
        zp = zp_t[0:1, 0:GPC]
        nc.tensor.matmul(zp, ones84[:], qsb[:], start=True, stop=True)
        zout = smpool.tile([1, GPC], F32, tag="zout")
        nc.scalar.activation(zout[:], zp, AF.Relu, bias=float(tail_bias))
        nc.sync.dma_start(out_d.rearrange("(o g) -> o g", o=1), zout[:])

    nc.compile()
    return nc


def _core_inputs(pre, c):
    m = {
        'xT': np.ascontiguousarray(pre['x_aug'][:, c * NB:(c + 1) * NB]),
        'maskc': pre['maskc'], 'lhs92': pre['lhs92'], 'ident': pre['ident'],
    }
    for l in range(DEPTH):
        m[f'E{l}'] = np.ascontiguousarray(
            np.transpose(pre['Es'][l][c * GPC:(c + 1) * GPC], (1, 0, 2))
            .reshape(NPG, NB))
        m[f'CW{l}'] = pre['CW'][l]
    for l in range(2):
        if np.any(pre['bl'][l] != 0):
            m[f'bb{l}'] = np.ascontiguousarray(
                np.tile(pre['bl'][l][None, :], (NPG, 1)).astype(np.float16))
    return m


def kernel(**inputs):
    pre = _host_preprocess(inputs)
    use_bias = tuple(bool(np.any(pre['bl'][l] != 0)) for l in range(2))
    nc = _build_program(pre['tail_bias'], use_bias)
    in_maps = [_core_inputs(pre, c) for c in range(NC_CORES)]
    res = run_bass_kernel_spmd(nc, in_maps, list(range(NC_CORES)))
    out = np.concatenate([np.asarray(res.results[c]['out'])
                          for c in range(NC_CORES)])
    return out.reshape(B, 1).astype(np.float32)
